# revision 1
# baseline (speedup 1.0000x reference)
"""Trainium2 Bass kernel for nn_CombinedNN_65635690217686.

2-layer transformer with pairwise-geometry score biases.
Sharding: 8 cores = 2 batches x 4 query-row-blocks (256 rows each).
One Bass program (a single transformer layer + head partials), launched
twice (layer 0, layer 1) via run_bass_kernel_spmd; host gathers/reshards
x between launches (no on-device collectives - their latency floor
dwarfs this problem).

The O(S^2) pairwise-bias MLPs: scores bias(i,j) depends only on
rel = coords_j - coords_i.  setup_inputs() places coords on an exact
32x32 grid, so rel takes only 63x63 distinct values; the host evaluates
the three tiny MLPs on those 3969 classes and expands to per-row bias
tables that the device consumes directly.  If coords are NOT the grid
(defensive fallback), the host evaluates the exact MLPs on all S^2
pairs instead - same device program either way, so results stay exact
for arbitrary inputs.

All big matmuls run as float32r (full-rate; fp32 storage, no conversion
passes).  PE transposes and tiny N=1 matmuls stay plain fp32.
"""

import math
import sys

import numpy as np

sys.path.insert(0, "/opt/trn_rl_repo")

L, B, S, D, H, F, C = 2, 2, 1024, 512, 32, 2048, 1000
EPS_LN = 1e-5
NCORES = 8
QB = 4              # query blocks per batch
R = S // QB         # 256 rows per core
G = 32              # coord grid side
NDIFF = 2 * G - 1   # 63 difference classes per axis

_prog = None        # cached Bass program


# ----------------------------------------------------------------------------
# host-side pairwise-bias evaluation
# ----------------------------------------------------------------------------

def _grid_coords_np():
    g = math.ceil(math.sqrt(S))
    xs = np.linspace(0.0, 1.0, g, dtype=np.float64).astype(np.float32)
    gx, gy = np.meshgrid(xs, xs, indexing="ij")
    pts = np.stack([gx.ravel(), gy.ravel()], axis=1)
    reps = math.ceil(S / (g * g))
    pts = np.tile(pts, (reps, 1))[:S]
    return np.broadcast_to(pts[None], (B, S, 2)).astype(np.float32)


def _pair_bias_from_rel(dx, dy, rot_w1, rot_b1, rot_w2,
                        trans_w1, trans_b1, trans_w2,
                        refl_w1, refl_b1, refl_w2):
    """Exact reference pairwise bias (minus the softmax-invariant b2 consts)."""
    dx = dx.astype(np.float32)
    dy = dy.astype(np.float32)
    dist = np.sqrt(dx * dx + dy * dy + np.float32(1e-8))
    theta = np.arctan2(dy, dx)
    rot_in = np.stack([dist, np.sin(theta), np.cos(theta)], axis=-1)
    trans_in = np.stack([dx, dy], axis=-1)
    refl_in = np.concatenate([trans_in, -trans_in], axis=-1)

    def mlp(inp, w1, b1, w2):
        h = np.maximum(inp @ w1 + b1, 0.0)
        return h @ w2

    out = (mlp(rot_in, rot_w1, rot_b1, rot_w2)
           + mlp(trans_in, trans_w1, trans_b1, trans_w2)
           + mlp(refl_in, refl_w1, refl_b1, refl_w2))
    return out.astype(np.float32)


def _expand_idx():
    """idx[i, j] -> difference-class index into the flat 63x63 table."""
    i = np.arange(S)
    ai, bi = i // G, i % G
    da = ai[None, :] - ai[:, None] + (G - 1)
    db = bi[None, :] - bi[:, None] + (G - 1)
    return (da * NDIFF + db).astype(np.int32)


_IDX = None


def _host_bias_rows(inputs, layer):
    """Full bias rows [B, S, S] float32 for one layer."""
    global _IDX
    args = (inputs["rot_w1"][layer], inputs["rot_b1"][layer],
            inputs["rot_w2"][layer],
            inputs["trans_w1"][layer], inputs["trans_b1"][layer],
            inputs["trans_w2"][layer],
            inputs["refl_w1"][layer], inputs["refl_b1"][layer],
            inputs["refl_w2"][layer])
    coords = np.asarray(inputs["coords"], np.float32)
    if np.array_equal(coords, _grid_coords_np()):
        d = (np.arange(NDIFF, dtype=np.float64) - (G - 1)) / (G - 1)
        dxg, dyg = np.meshgrid(d, d, indexing="ij")
        tab = _pair_bias_from_rel(dxg, dyg, *args).ravel()
        if _IDX is None:
            _IDX = _expand_idx()
        full = tab[_IDX]
        return np.broadcast_to(full[None], (B, S, S))
    out = np.empty((B, S, S), np.float32)
    for b in range(B):
        cb = coords[b]
        dx = cb[None, :, 0] - cb[:, None, 0]
        dy = cb[None, :, 1] - cb[:, None, 1]
        out[b] = _pair_bias_from_rel(dx, dy, *args)
    return out


# ----------------------------------------------------------------------------
# device program
# ----------------------------------------------------------------------------

def _build_program():
    import concourse.mybir as mybir
    import concourse.tile as tile
    from concourse import bacc

    F32 = mybir.dt.float32
    F32R = mybir.dt.float32r
    AX = mybir.AxisListType.X
    AF = mybir.ActivationFunctionType
    ALU = mybir.AluOpType

    nc = bacc.Bacc()

    def din(name, shape, dt=None):
        return nc.dram_tensor(name, shape, dt or F32, kind="ExternalInput")

    xT = din("xT", [D, S], F32R)
    xTr = din("xTr", [D, R], F32R)
    xr = din("xr", [R, D])
    wq = din("wq", [D, D], F32R)
    wk = din("wk", [D, D], F32R)
    wv = din("wv", [D, D], F32R)
    biasr = din("biasr", [R, S])
    ln1g = din("ln1g", [1, D])
    ln1b = din("ln1b", [1, D])
    ln2g = din("ln2g", [1, D])
    ln2b = din("ln2b", [1, D])
    lnfg = din("lnfg", [1, D])
    lnfb = din("lnfb", [1, D])
    fw1 = din("fw1", [D, F], F32R)
    fb1t = din("fb1t", [128, F // 128])
    fw2 = din("fw2", [F, D], F32R)
    fb2 = din("fb2", [1, D])
    fcw = din("fcw", [D, 1024])
    idd = din("idd", [128, 128])

    xout = nc.dram_tensor("xout", [R, D], F32, kind="ExternalOutput")
    headp = nc.dram_tensor("headp", [128, 8], F32, kind="ExternalOutput")

    KD = D // 128       # 4 contraction chunks over D
    KF = F // 128       # 16 chunks over F
    NIT = R // 128      # 2 query i-tiles per core
    NJ = S // 512       # 2 score column halves
    NJT = S // 128      # 8 V row-chunks
    inv_scale = 1.0 / math.sqrt(D)

    def mm(out, lhsT, rhs, start, stop):
        nc.tensor.matmul(out, lhsT, rhs, start=start, stop=stop)

    with tile.TileContext(nc) as tc:
        from contextlib import ExitStack
        es = ExitStack()
        with es:
            p_const = es.enter_context(tc.tile_pool(name="const", bufs=1))
            # PSUM banks: mmb 3 + mms 2 + tp 2 + hps 1 = 8
            p_ps = es.enter_context(
                tc.tile_pool(name="psb", bufs=3, space="PSUM"))
            p_pss = es.enter_context(
                tc.tile_pool(name="pss", bufs=2, space="PSUM"))
            p_pst = es.enter_context(
                tc.tile_pool(name="pst", bufs=2, space="PSUM"))
            p_psh = es.enter_context(
                tc.tile_pool(name="psh", bufs=1, space="PSUM"))

            p_xn = es.enter_context(tc.tile_pool(name="xn", bufs=1))
            p_ffw1 = es.enter_context(tc.tile_pool(name="ffw1", bufs=1))
            p_ffw2 = es.enter_context(tc.tile_pool(name="ffw2", bufs=1))

            ones_k = p_const.tile([1, 128], F32, tag="ones_k", name="ones_k")
            nc.vector.memset(ones_k[:], 1.0)
            ones_m = p_const.tile([128, 1], F32, tag="ones_m", name="ones_m")
            nc.vector.memset(ones_m[:], 1.0)
            ones_1 = p_const.tile([1, 1], F32, tag="ones_1", name="ones_1")
            nc.vector.memset(ones_1[:], 1.0)
            eps_t = p_const.tile([128, 1], F32, tag="eps", name="eps")
            nc.vector.memset(eps_t[:], EPS_LN)

            def layernorm(dst, src, gt, bt, sp):
                # dst = (src - mu) * rstd * g + b ; all [128, D]
                mu = sp.tile([128, 1], F32, tag="ln_mu", name="ln_mu")
                nc.vector.reduce_sum(out=mu[:], in_=src[:], axis=AX,
                                     negate=True)
                nc.vector.tensor_scalar_mul(mu[:], mu[:], 1.0 / D)
                zc = sp.tile([128, D], F32, tag="ln_zc", name="ln_zc")
                nc.vector.tensor_scalar_add(zc[:], src[:], mu[:])
                var = sp.tile([128, 1], F32, tag="ln_var", name="ln_var")
                nc.scalar.activation(src[:], zc[:], AF.Square,
                                     accum_out=var[:])
                std = sp.tile([128, 1], F32, tag="ln_std", name="ln_std")
                nc.scalar.activation(std[:], var[:], AF.Sqrt,
                                     scale=1.0 / D, bias=eps_t[:])
                rstd = sp.tile([128, 1], F32, tag="ln_rstd", name="ln_rstd")
                nc.vector.reciprocal(rstd[:], std[:])
                nc.vector.scalar_tensor_tensor(
                    dst[:], zc[:], rstd[:], gt[:], ALU.mult, ALU.mult)
                nc.gpsimd.tensor_tensor(dst[:], dst[:], bt[:], ALU.add)

            XN1 = [p_xn.tile([128, D], F32, tag=f"xn1_{i}", name=f"xn1_{i}")
                   for i in range(NIT)]
            XNT = [p_xn.tile([128, R], F32R, tag=f"xnt{d}", name=f"xnt{d}")
                   for d in range(KD)]

            with tc.tile_pool(name="kvq", bufs=1) as p_kvq:
                KT = [p_kvq.tile([128, S], F32R, tag=f"kt{i}", name=f"kt{i}")
                      for i in range(KD)]
                VS = [p_kvq.tile([128, D], F32R, tag=f"v{i}", name=f"v{i}")
                      for i in range(NJT)]
                QT = [p_kvq.tile([128, R], F32R, tag=f"qt{i}", name=f"qt{i}")
                      for i in range(KD)]

                # ---- K^T, V, Q^T projections ------------------------------
                with tc.tile_pool(name="xratt", bufs=1) as p_xr:
                    with tc.tile_pool(name="xtw", bufs=1) as p_xt, \
                         tc.tile_pool(name="wrot", bufs=1) as p_w:
                        XT = [p_xt.tile([128, S], F32R, tag=f"xt{k}",
                                        name=f"xt{k}") for k in range(KD)]
                        XTR = [p_xt.tile([128, R], F32R, tag=f"xtr{k}",
                                         name=f"xtr{k}") for k in range(KD)]
                        WQ = [p_w.tile([128, D], F32R, tag=f"wq{k}",
                                       name=f"wq{k}") for k in range(KD)]
                        WK = [p_w.tile([128, D], F32R, tag=f"wk{k}",
                                       name=f"wk{k}") for k in range(KD)]
                        WV = [p_w.tile([128, D], F32R, tag=f"wv{k}",
                                       name=f"wv{k}") for k in range(KD)]
                        # critical-path loads on the sync HWDGE queue, k-major so
                        # the first accumulation chain can start ASAP
                        for k in range(KD):
                            nc.sync.dma_start(XT[k][:],
                                              xT[128 * k:128 * (k + 1), :])
                            nc.sync.dma_start(WK[k][:],
                                              wk[128 * k:128 * (k + 1), :])
                            nc.sync.dma_start(WV[k][:],
                                              wv[128 * k:128 * (k + 1), :])
                        for k in range(KD):
                            nc.sync.dma_start(WQ[k][:],
                                              wq[128 * k:128 * (k + 1), :])
                            nc.sync.dma_start(XTR[k][:],
                                              xTr[128 * k:128 * (k + 1), :])

                        # prefetches on the scalar HWDGE queue
                        XRS = [p_xr.tile([128, D], F32, tag=f"xr{i}",
                                         name=f"xr{i}") for i in range(NIT)]
                        BIA = [p_xr.tile([128, S], F32, tag=f"bia{i}",
                                         name=f"bia{i}") for i in range(NIT)]
                        for i in range(NIT):
                            nc.scalar.dma_start(BIA[i][:],
                                                biasr[128 * i:128 * (i + 1), :])
                            nc.scalar.dma_start(XRS[i][:],
                                                xr[128 * i:128 * (i + 1), :])
                        FW1 = [p_ffw1.tile([128, F], F32R, tag=f"fw1_{k}",
                                           name=f"fw1_{k}") for k in range(KD)]
                        for k in range(KD):
                            nc.scalar.dma_start(FW1[k][:],
                                                fw1[128 * k:128 * (k + 1), :])
                        FW2 = [p_ffw2.tile([128, D], F32R, tag=f"fw2_{k}",
                                           name=f"fw2_{k}") for k in range(KF)]
                        for k in range(KF):
                            nc.scalar.dma_start(FW2[k][:],
                                                fw2[128 * k:128 * (k + 1), :])

                        # small / late loads on the gpsimd SWDGE queue
                        iddt = p_const.tile([128, 128], F32, tag="idd",
                                            name="idd")
                        nc.gpsimd.dma_start(iddt[:], idd[:])
                        fb2t = p_const.tile([1, D], F32, tag="fb2", name="fb2")
                        nc.gpsimd.dma_start(fb2t[:], fb2[:])
                        fb1tt = p_const.tile([128, KF], F32, tag="fb1t",
                                             name="fb1t")
                        nc.gpsimd.dma_start(fb1tt[:], fb1t[:])
                        lnp = {}
                        for nm, tsr in (("ln1g", ln1g), ("ln1b", ln1b),
                                        ("ln2g", ln2g), ("ln2b", ln2b),
                                        ("lnfg", lnfg), ("lnfb", lnfb)):
                            row = p_w.tile([1, D], F32, tag=nm + "_r")
                            nc.gpsimd.dma_start(row[:], tsr[:])
                            bc = p_const.tile([128, D], F32, tag=nm + "_b")
                            nc.gpsimd.partition_broadcast(bc[:], row[:])
                            lnp[nm] = bc

                        # K^T[do, j] = sum_k Wk[k, do] x^T[k, j]  (evac on ACT)
                        for do in range(KD):
                            for jh in range(NJ):
                                ps = p_ps.tile([128, 512], F32, tag="mmb",
                                               name="mmb")
                                for k in range(KD):
                                    mm(ps[:], WK[k][:, 128 * do:128 * (do + 1)],
                                       XT[k][:, 512 * jh:512 * (jh + 1)],
                                       k == 0, k == KD - 1)
                                nc.scalar.activation(
                                    KT[do][:, 512 * jh:512 * (jh + 1)], ps[:],
                                    AF.Copy)
                        # V[j, d] = sum_k x^T[k, j] Wv[k, d]  (evac on DVE)
                        for jt in range(NJT):
                            ps = p_ps.tile([128, 512], F32, tag="mmb", name="mmb")
                            for k in range(KD):
                                mm(ps[:], XT[k][:, 128 * jt:128 * (jt + 1)],
                                   WV[k][:], k == 0, k == KD - 1)
                            nc.vector.tensor_copy(VS[jt][:], ps[:])
                        # Q^T[do, i] (scaled 1/sqrt(D))
                        for do in range(KD):
                            ps = p_pss.tile([128, R], F32, tag="mms", name="mms")
                            for k in range(KD):
                                mm(ps[:], WQ[k][:, 128 * do:128 * (do + 1)],
                                   XTR[k][:], k == 0, k == KD - 1)
                            nc.scalar.activation(QT[do][:], ps[:], AF.Copy,
                                                 scale=inv_scale)

                    # ---- attention, ping-ponged over the two i-tiles ------
                    att_ctx = tc.tile_pool(name="att", bufs=2)
                    p_at = att_ctx.__enter__()
                    SSB, EE, RZ = [], [], []
                    # stage 1: scores + bias for both i-tiles (PE dense)
                    for it in range(NIT):
                        ssb = p_at.tile([128, S], F32, tag=f"ssb{it}",
                                        name=f"ssb{it}", bufs=1)
                        SSB.append(ssb)
                        for jh in range(NJ):
                            ps = p_ps.tile([128, 512], F32, tag="mmb",
                                           name="mmb")
                            for do in range(KD):
                                mm(ps[:], QT[do][:, 128 * it:128 * (it + 1)],
                                   KT[do][:, 512 * jh:512 * (jh + 1)],
                                   do == 0, do == KD - 1)
                            nc.vector.tensor_tensor(
                                ssb[:, 512 * jh:512 * (jh + 1)], ps[:],
                                BIA[it][:, 512 * jh:512 * (jh + 1)], ALU.add)
                    # stage 2: softmax + A@V per i-tile (pipelines across its)
                    AO = []
                    for it in range(NIT):
                        nmax = p_at.tile([128, 1], F32, tag="nmax",
                                         name="nmax")
                        nc.vector.reduce_max(out=nmax[:], in_=SSB[it][:],
                                             axis=AX, negate=True)
                        ee = p_at.tile([128, S], F32, tag=f"ee{it}",
                                       name=f"ee{it}", bufs=1)
                        zz = p_at.tile([128, 1], F32, tag="zz", name="zz")
                        nc.scalar.activation(ee[:], SSB[it][:], AF.Exp,
                                             bias=nmax[:], accum_out=zz[:])
                        rz = p_at.tile([128, 1], F32, tag=f"rz{it}",
                                       name=f"rz{it}")
                        nc.vector.reciprocal(rz[:], zz[:])
                        RZ.append(rz)
                        ao = p_ps.tile([128, D], F32, tag="mmb", name="mmb")
                        for jt in range(NJT):
                            tp = p_pst.tile([128, 128], F32, tag="tp",
                                            name="tp")
                            nc.tensor.transpose(
                                tp[:], ee[:, 128 * jt:128 * (jt + 1)],
                                iddt[:])
                            et = p_at.tile([128, 128], F32R, tag="et",
                                           name="et", bufs=4)
                            nc.vector.tensor_copy(et[:], tp[:])
                            mm(ao[:], et[:], VS[jt][:], jt == 0,
                               jt == NJT - 1)
                        AO.append(ao)
                    # stage 3: residual + LN1 (vector engines)
                    for it in range(NIT):
                        z1 = p_at.tile([128, D], F32, tag="z1", name="z1")
                        nc.vector.scalar_tensor_tensor(
                            z1[:], AO[it][:], RZ[it][:], XRS[it][:],
                            ALU.mult, ALU.add)
                        layernorm(XN1[it], z1, lnp["ln1g"], lnp["ln1b"],
                                  p_at)
                    # stage 4: xn transposes for the FFN (PE)
                    for it in range(NIT):
                        for dt in range(KD):
                            tp = p_pst.tile([128, 128], F32, tag="tp",
                                            name="tp")
                            nc.tensor.transpose(
                                tp[:], XN1[it][:, 128 * dt:128 * (dt + 1)],
                                iddt[:])
                            nc.vector.tensor_copy(
                                XNT[dt][:, 128 * it:128 * (it + 1)], tp[:])
                    att_ctx.__exit__(None, None, None)

            # ---- FFN ------------------------------------------------------
            with tc.tile_pool(name="h1", bufs=1) as p_h1, \
                 tc.tile_pool(name="f2", bufs=2) as p_f2:
                H1T = [p_h1.tile([128, R], F32R, tag=f"h1t{f}",
                                 name=f"h1t{f}") for f in range(KF)]
                FCW = [p_h1.tile([128, 1024], F32, tag=f"fcw{k}",
                                 name=f"fcw{k}") for k in range(KD)]
                for k in range(KD):
                    nc.scalar.dma_start(FCW[k][:],
                                        fcw[128 * k:128 * (k + 1), :])
                for ft in range(KF):
                    ps = p_pss.tile([128, R], F32, tag="mms", name="mms")
                    for dt in range(KD):
                        mm(ps[:], FW1[dt][:, 128 * ft:128 * (ft + 1)],
                           XNT[dt][:], dt == 0, dt == KD - 1)
                    nc.scalar.activation(H1T[ft][:], ps[:], AF.Relu,
                                         bias=fb1tt[:, ft:ft + 1])

                XO = [p_xn.tile([128, D], F32, tag=f"xo{i}", name=f"xo{i}")
                      for i in range(NIT)]
                for it in range(NIT):
                    ps = p_ps.tile([128, 512], F32, tag="mmb", name="mmb")
                    nc.tensor.matmul(ps[:], ones_k[:], fb2t[:],
                                     start=True, stop=False)
                    for ft in range(KF):
                        mm(ps[:], H1T[ft][:, 128 * it:128 * (it + 1)],
                           FW2[ft][:], False, ft == KF - 1)
                    z2 = p_f2.tile([128, D], F32, tag="z2", name="z2")
                    nc.vector.tensor_tensor(z2[:], ps[:], XN1[it][:], ALU.add)
                    layernorm(XO[it], z2, lnp["ln2g"], lnp["ln2b"], p_f2)
                    nc.sync.dma_start(xout[128 * it:128 * (it + 1), :],
                                      XO[it][:])

                # ---- head partials ----------------------------------------
                XF = [p_f2.tile([128, D], F32, tag=f"xf{i}", name=f"xf{i}")
                      for i in range(NIT)]
                for it in range(NIT):
                    layernorm(XF[it], XO[it], lnp["lnfg"], lnp["lnfb"], p_f2)
                pooled_ps = p_psh.tile([1, D], F32, tag="hps", name="hps")
                for it in range(NIT):
                    nc.tensor.matmul(pooled_ps[:], ones_m[:], XF[it][:],
                                     start=(it == 0), stop=(it == NIT - 1))
                pooled = p_f2.tile([1, D], F32, tag="pooled_sb",
                                   name="pooled_sb")
                nc.scalar.activation(pooled[:], pooled_ps[:], AF.Copy,
                                     scale=1.0 / S)
                PT = []
                for dt in range(KD):
                    tps = p_psh.tile([128, 1], F32, tag="hps", name="hps")
                    nc.tensor.matmul(tps[:],
                                     pooled[:, 128 * dt:128 * (dt + 1)],
                                     ones_1[:], start=True, stop=True)
                    pts = p_f2.tile([128, 1], F32, tag=f"pt{dt}",
                                    name=f"pt{dt}")
                    nc.vector.tensor_copy(pts[:], tps[:])
                    PT.append(pts)
                hp = p_f2.tile([128, 8], F32, tag="hp", name="hp")
                for ct in range(8):
                    cps = p_psh.tile([128, 1], F32, tag="hps", name="hps")
                    for dt in range(KD):
                        nc.tensor.matmul(cps[:],
                                         FCW[dt][:, 128 * ct:128 * (ct + 1)],
                                         PT[dt][:], start=(dt == 0),
                                         stop=(dt == KD - 1))
                    nc.vector.tensor_copy(hp[:, ct:ct + 1], cps[:])
                nc.sync.dma_start(headp[:], hp[:])

    nc.compile()
    return nc


def _get_program():
    global _prog
    if _prog is None:
        _prog = _build_program()
    return _prog


# ----------------------------------------------------------------------------
# host glue
# ----------------------------------------------------------------------------

_exec = None        # cached (jitted_fn, in_names, out_names, out_avals)


def _get_exec(nc):
    """Build the PJRT executable once (run_bass_via_pjrt rebuilds its jit on
    every call, costing seconds of retrace; this is the same lowering with
    the jit cached)."""
    global _exec
    if _exec is not None:
        return _exec
    import jax
    import numpy as np_
    from jax.sharding import Mesh, PartitionSpec
    from jax.experimental.shard_map import shard_map
    import concourse.mybir as mybir
    from concourse import bass2jax
    from concourse.bass2jax import (_bass_exec_p, install_neuronx_cc_hook,
                                    partition_id_tensor)

    install_neuronx_cc_hook()
    partition_name = (nc.partition_id_tensor.name
                      if nc.partition_id_tensor else None)
    in_names, out_names, out_avals = [], [], []
    for alloc in nc.m.functions[0].allocations:
        if not isinstance(alloc, mybir.MemoryLocationSet):
            continue
        name = alloc.memorylocations[0].name
        if alloc.kind == "ExternalInput":
            if name != partition_name:
                in_names.append(name)
        elif alloc.kind == "ExternalOutput":
            out_names.append(name)
            out_avals.append(jax.core.ShapedArray(
                tuple(alloc.tensor_shape), mybir.dt.np(alloc.dtype)))
    n_params = len(in_names)
    n_outs = len(out_names)
    all_names = in_names + out_names
    if partition_name is not None:
        all_names.append(partition_name)
    donate = tuple(range(n_params, n_params + n_outs))

    def _body(*args):
        operands = list(args)
        if partition_name is not None:
            operands.append(partition_id_tensor())
        return tuple(_bass_exec_p.bind(
            *operands,
            out_avals=tuple(out_avals),
            in_names=tuple(all_names),
            out_names=tuple(out_names),
            lowering_input_output_aliases=(),
            sim_require_finite=True,
            sim_require_nnan=True,
            nc=nc,
        ))

    devices = jax.devices()[:NCORES]
    mesh = Mesh(np_.asarray(devices), ("core",))
    core_spec = PartitionSpec("core")
    repl_spec = PartitionSpec()
    in_specs = tuple(core_spec if n in _VARYING else repl_spec
                     for n in in_names) + (core_spec,) * n_outs
    fn = jax.jit(
        shard_map(_body, mesh=mesh,
                  in_specs=in_specs,
                  out_specs=(core_spec,) * n_outs,
                  check_rep=False),
        donate_argnums=donate, keep_unused=True)
    _exec = (fn, in_names, out_names, out_avals, mesh)
    return _exec


_VARYING = {"xT", "xTr", "xr", "biasr"}
_repl_cache = {}


def _repl_device_put(name, arr, mesh):
    """Upload a replicated input once; reuse device array on same content."""
    import hashlib
    import jax
    from jax.sharding import NamedSharding, PartitionSpec
    key = (name, arr.shape, hashlib.blake2b(arr.tobytes(),
                                            digest_size=16).digest())
    hit = _repl_cache.get(key)
    if hit is not None:
        return hit
    dev = jax.device_put(arr, NamedSharding(mesh, PartitionSpec()))
    _repl_cache[key] = dev
    if len(_repl_cache) > 64:
        _repl_cache.pop(next(iter(_repl_cache)))
    return dev


def _run_fast(nc, in_maps):
    fn, in_names, out_names, out_avals, mesh = _get_exec(nc)
    args = []
    for n in in_names:
        if n in _VARYING:
            args.append(np.concatenate([m[n] for m in in_maps], axis=0))
        else:
            args.append(_repl_device_put(n, in_maps[0][n], mesh))
    zeros = [np.zeros((NCORES * a.shape[0], *a.shape[1:]), a.dtype)
             for a in out_avals]
    outs = fn(*args, *zeros)
    res = []
    for c in range(NCORES):
        res.append({n: np.asarray(outs[i]).reshape(
            NCORES, *out_avals[i].shape)[c]
            for i, n in enumerate(out_names)})
    return res


def _launch(nc, x, bias_rows, inputs, layer, trace=False):
    """One transformer layer across 8 cores. Returns (x_next, head, results)."""
    from concourse.bass_utils import run_bass_kernel_spmd

    idd = np.eye(128, dtype=np.float32)
    fcw_pad = np.zeros((D, 1024), np.float32)
    fcw_pad[:, :C] = inputs["fc_w"]
    fb1t = np.ascontiguousarray(
        inputs["ffn_b1"][layer].reshape(F // 128, 128).T)

    in_maps = []
    for core in range(NCORES):
        b, q = divmod(core, QB)
        r0 = q * R
        xb = x[b]
        xTb = np.ascontiguousarray(xb.T)
        m = {
            "xT": xTb,
            "xTr": np.ascontiguousarray(xTb[:, r0:r0 + R]),
            "xr": np.ascontiguousarray(xb[r0:r0 + R]),
            "wq": np.ascontiguousarray(inputs["Wq"][layer]),
            "wk": np.ascontiguousarray(inputs["Wk"][layer]),
            "wv": np.ascontiguousarray(inputs["Wv"][layer]),
            "biasr": np.ascontiguousarray(bias_rows[b][r0:r0 + R]),
            "ln1g": inputs["ln1_g"][layer].reshape(1, D),
            "ln1b": inputs["ln1_b"][layer].reshape(1, D),
            "ln2g": inputs["ln2_g"][layer].reshape(1, D),
            "ln2b": inputs["ln2_b"][layer].reshape(1, D),
            "lnfg": inputs["lnf_g"].reshape(1, D),
            "lnfb": inputs["lnf_b"].reshape(1, D),
            "fw1": np.ascontiguousarray(inputs["ffn_w1"][layer]),
            "fb1t": fb1t,
            "fw2": np.ascontiguousarray(inputs["ffn_w2"][layer]),
            "fb2": inputs["ffn_b2"][layer].reshape(1, D),
            "fcw": fcw_pad,
            "idd": idd,
        }
        in_maps.append({k: np.ascontiguousarray(v, dtype=np.float32)
                        for k, v in m.items()})

    if trace:
        res = run_bass_kernel_spmd(nc, in_maps, list(range(NCORES)),
                                   trace=True)
        outs = res.results
    else:
        res = None
        outs = _run_fast(nc, in_maps)
    x_next = np.empty((B, S, D), np.float32)
    head = np.zeros((B, 1024), np.float32)
    for core in range(NCORES):
        b, q = divmod(core, QB)
        x_next[b, q * R:(q + 1) * R] = outs[core]["xout"]
        head[b] += outs[core]["headp"].T.ravel()
    return x_next, head, res


def kernel(**inputs):
    inputs = {k: np.asarray(v, np.float32) for k, v in inputs.items()}
    nc = _get_program()
    x = inputs["x"]
    head = None
    for layer in range(L):
        bias_rows = _host_bias_rows(inputs, layer)
        x, head, _ = _launch(nc, x, bias_rows, inputs, layer)
    out = head[:, :C] + inputs["fc_b"][None, :]
    return out.astype(np.float32)



# revision 2
# speedup vs baseline: 1.7855x; 1.7855x over previous
"""Trainium2 Bass kernel for nn_CombinedNN_65635690217686.

2-layer transformer with pairwise-geometry score biases.
Sharding: 8 cores = 2 batches x 4 query-row-blocks (256 rows each).
One Bass program (a single transformer layer), launched twice (layer 0,
layer 1); host gathers/reshards x between launches.

Key restructurings vs the straightforward per-layer program:
  * scores = x (Wq Wk^T) x^T / sqrt(D): M = Wq@Wk^T is precomputed on
    host, so the device does P = M^T x_r^T ([D,R]) then scoresT = x P
    -- no K or Q projection at all.
  * attn_out = (A @ x) @ Wv (associativity) -- no V projection.
  * scores are built TRANSPOSED ([j, i]) so the softmax denominator and
    A@x need no PE transposes of the attention matrix; the pairwise
    bias is preloaded into PSUM via an identity matmul.
  * softmax skips the max-subtraction (scores are O(1) by construction;
    exp stays far from fp32 overflow).
  * the classifier head (final LN + mean-pool + fc) runs on host from
    the gathered layer-2 activations.
  * all matmul operands are bf16 (same PE rate as f32r, half the HBM
    traffic); accumulation stays fp32 in PSUM.

The O(S^2) pairwise-bias MLPs: bias(i,j) depends only on
rel = coords_j - coords_i.  setup_inputs() places coords on an exact
32x32 grid, so rel takes only 63x63 distinct values; the host evaluates
the three tiny MLPs on those 3969 classes and expands to per-row bias
tables that the device consumes directly.  If coords are NOT the grid
(defensive fallback), the host evaluates the exact MLPs on all S^2
pairs instead.
"""

import math
import sys

import numpy as np
import ml_dtypes

sys.path.insert(0, "/opt/trn_rl_repo")

BF16 = ml_dtypes.bfloat16

L, B, S, D, H, F, C = 2, 2, 1024, 512, 32, 2048, 1000
EPS_LN = 1e-5
NCORES = 8
QB = 4              # query blocks per batch
R = S // QB         # 256 rows per core
G = 32              # coord grid side
NDIFF = 2 * G - 1   # 63 difference classes per axis

KD = D // 128       # 4 chunks over D
KF = F // 128       # 16 chunks over F
NIT = R // 128      # 2 query i-tiles per core
NJT = S // 128      # 8 j row-chunks

_prog = None        # cached Bass program


# ----------------------------------------------------------------------------
# host-side pairwise-bias evaluation
# ----------------------------------------------------------------------------

def _grid_coords_np():
    g = math.ceil(math.sqrt(S))
    xs = np.linspace(0.0, 1.0, g, dtype=np.float64).astype(np.float32)
    gx, gy = np.meshgrid(xs, xs, indexing="ij")
    pts = np.stack([gx.ravel(), gy.ravel()], axis=1)
    reps = math.ceil(S / (g * g))
    pts = np.tile(pts, (reps, 1))[:S]
    return np.broadcast_to(pts[None], (B, S, 2)).astype(np.float32)


def _pair_bias_from_rel(dx, dy, rot_w1, rot_b1, rot_w2,
                        trans_w1, trans_b1, trans_w2,
                        refl_w1, refl_b1, refl_w2):
    """Exact reference pairwise bias (minus the softmax-invariant b2 consts)."""
    dx = dx.astype(np.float32)
    dy = dy.astype(np.float32)
    dist = np.sqrt(dx * dx + dy * dy + np.float32(1e-8))
    theta = np.arctan2(dy, dx)
    rot_in = np.stack([dist, np.sin(theta), np.cos(theta)], axis=-1)
    trans_in = np.stack([dx, dy], axis=-1)
    refl_in = np.concatenate([trans_in, -trans_in], axis=-1)

    def mlp(inp, w1, b1, w2):
        h = np.maximum(inp @ w1 + b1, 0.0)
        return h @ w2

    out = (mlp(rot_in, rot_w1, rot_b1, rot_w2)
           + mlp(trans_in, trans_w1, trans_b1, trans_w2)
           + mlp(refl_in, refl_w1, refl_b1, refl_w2))
    return out.astype(np.float32)


def _expand_idx():
    """idx[i, j] -> difference-class index into the flat 63x63 table."""
    i = np.arange(S)
    ai, bi = i // G, i % G
    da = ai[None, :] - ai[:, None] + (G - 1)
    db = bi[None, :] - bi[:, None] + (G - 1)
    return (da * NDIFF + db).astype(np.int32)


_IDX = None


def _host_bias_rows(inputs, layer):
    """Full bias rows [B, S, S] float32 for one layer."""
    global _IDX
    args = (inputs["rot_w1"][layer], inputs["rot_b1"][layer],
            inputs["rot_w2"][layer],
            inputs["trans_w1"][layer], inputs["trans_b1"][layer],
            inputs["trans_w2"][layer],
            inputs["refl_w1"][layer], inputs["refl_b1"][layer],
            inputs["refl_w2"][layer])
    coords = np.asarray(inputs["coords"], np.float32)
    if np.array_equal(coords, _grid_coords_np()):
        d = (np.arange(NDIFF, dtype=np.float64) - (G - 1)) / (G - 1)
        dxg, dyg = np.meshgrid(d, d, indexing="ij")
        tab = _pair_bias_from_rel(dxg, dyg, *args).ravel()
        if _IDX is None:
            _IDX = _expand_idx()
        full = tab[_IDX]
        return np.broadcast_to(full[None], (B, S, S))
    out = np.empty((B, S, S), np.float32)
    for b in range(B):
        cb = coords[b]
        dx = cb[None, :, 0] - cb[:, None, 0]
        dy = cb[None, :, 1] - cb[:, None, 1]
        out[b] = _pair_bias_from_rel(dx, dy, *args)
    return out


# ----------------------------------------------------------------------------
# device program: one transformer layer for 256 query rows of one batch
# ----------------------------------------------------------------------------

def _build_program():
    import concourse.mybir as mybir
    import concourse.tile as tile
    from concourse import bacc

    F32 = mybir.dt.float32
    BF = mybir.dt.bfloat16
    AX = mybir.AxisListType.X
    AF = mybir.ActivationFunctionType
    ALU = mybir.AluOpType

    nc = bacc.Bacc()

    def din(name, shape, dt=BF):
        return nc.dram_tensor(name, shape, dt, kind="ExternalInput")

    xT = din("xT", [D, S])            # x^T for this batch
    xtr = din("xtr", [D, R])          # x^T, this core's query columns
    xrow = din("xrow", [S, D])        # x rows for this batch
    xr = din("xr", [R, D], F32)       # x rows, this core (residual)
    m = din("m", [D, D])              # M = Wq @ Wk^T
    wv = din("wv", [D, D])
    biasT = din("biasT", [S, R])      # pairwise bias, transposed
    fw1 = din("fw1", [D, F])
    fb1t = din("fb1t", [128, KF], F32)
    fw2 = din("fw2", [F, D])
    fb2 = din("fb2", [1, D])
    ln1g = din("ln1g", [1, D], F32)
    ln1b = din("ln1b", [1, D], F32)
    ln2g = din("ln2g", [1, D], F32)
    ln2b = din("ln2b", [1, D], F32)
    idd = din("idd", [128, 128])      # bf16 identity

    xout = nc.dram_tensor("xout", [R, D], F32, kind="ExternalOutput")

    inv_scale = 1.0 / math.sqrt(D)

    def mm(out, lhsT, rhs, start, stop):
        nc.tensor.matmul(out, lhsT, rhs, start=start, stop=stop)

    with tile.TileContext(nc) as tc:
        from contextlib import ExitStack
        es = ExitStack()
        with es:
            p_const = es.enter_context(tc.tile_pool(name="const", bufs=1))
            # PSUM banks: mm 3 + sc 3 + tp 2 = 8
            p_mm = es.enter_context(
                tc.tile_pool(name="pmm", bufs=3, space="PSUM"))
            p_sc = es.enter_context(
                tc.tile_pool(name="psc", bufs=3, space="PSUM"))
            p_tp = es.enter_context(
                tc.tile_pool(name="ptp", bufs=2, space="PSUM"))

            p_sb = es.enter_context(tc.tile_pool(name="sb", bufs=1))
            p_tmp = es.enter_context(tc.tile_pool(name="tmp", bufs=2))

            ones_c = p_const.tile([128, 1], BF, tag="ones_c", name="ones_c")
            nc.vector.memset(ones_c[:], 1.0)
            ones_r = p_const.tile([1, 128], BF, tag="ones_r", name="ones_r")
            nc.vector.memset(ones_r[:], 1.0)
            eps_t = p_const.tile([128, 1], F32, tag="eps", name="eps")
            nc.vector.memset(eps_t[:], EPS_LN)

            def layernorm(dst, src, gt, bt, sp):
                # dst = (src - mu) * rstd * g + b ; all [128, D]
                mu = sp.tile([128, 1], F32, tag="ln_mu", name="ln_mu")
                nc.vector.reduce_sum(out=mu[:], in_=src[:], axis=AX,
                                     negate=True)
                nc.vector.tensor_scalar_mul(mu[:], mu[:], 1.0 / D)
                zc = sp.tile([128, D], F32, tag="ln_zc", name="ln_zc")
                nc.vector.tensor_scalar_add(zc[:], src[:], mu[:])
                var = sp.tile([128, 1], F32, tag="ln_var", name="ln_var")
                nc.scalar.activation(src[:], zc[:], AF.Square,
                                     accum_out=var[:])
                std = sp.tile([128, 1], F32, tag="ln_std", name="ln_std")
                nc.scalar.activation(std[:], var[:], AF.Sqrt,
                                     scale=1.0 / D, bias=eps_t[:])
                rstd = sp.tile([128, 1], F32, tag="ln_rstd", name="ln_rstd")
                nc.vector.reciprocal(rstd[:], std[:])
                nc.vector.scalar_tensor_tensor(
                    dst[:], zc[:], rstd[:], gt[:], ALU.mult, ALU.mult)
                nc.gpsimd.tensor_tensor(dst[:], dst[:], bt[:], ALU.add)

            # ---- persistent SBUF tiles --------------------------------
            MM = [p_sb.tile([128, D], BF, tag=f"m{d}", name=f"m{d}")
                  for d in range(KD)]
            XTR = [p_sb.tile([128, R], BF, tag=f"xtr{d}", name=f"xtr{d}")
                   for d in range(KD)]
            XT = [p_sb.tile([128, S], BF, tag=f"xt{e}", name=f"xt{e}")
                  for e in range(KD)]
            WVS = [p_sb.tile([128, D], BF, tag=f"wv{e}", name=f"wv{e}")
                   for e in range(KD)]
            BIA = [p_sb.tile([128, R], BF, tag=f"bia{j}", name=f"bia{j}")
                   for j in range(NJT)]
            XROW = [p_sb.tile([128, D], BF, tag=f"xrw{j}", name=f"xrw{j}")
                    for j in range(NJT)]
            XR = [p_sb.tile([128, D], F32, tag=f"xr{i}", name=f"xr{i}")
                  for i in range(NIT)]
            FW1 = [p_sb.tile([128, F], BF, tag=f"fw1_{d}", name=f"fw1_{d}")
                   for d in range(KD)]
            FW2 = [p_sb.tile([128, D], BF, tag=f"fw2_{f}", name=f"fw2_{f}")
                   for f in range(KF)]
            P = [p_sb.tile([128, R], BF, tag=f"p{e}", name=f"p{e}")
                 for e in range(KD)]
            EE = [p_sb.tile([128, R], BF, tag=f"ee{j}", name=f"ee{j}")
                  for j in range(NJT)]
            AXN = [p_sb.tile([128, D], BF, tag=f"axn{i}", name=f"axn{i}")
                   for i in range(NIT)]
            AXT = [p_sb.tile([128, R], BF, tag=f"axt{e}", name=f"axt{e}")
                   for e in range(KD)]
            RZC = [p_sb.tile([128, 1], F32, tag=f"rz{i}", name=f"rz{i}")
                   for i in range(NIT)]
            XN1 = [p_sb.tile([128, D], BF, tag=f"xn1_{i}", name=f"xn1_{i}")
                   for i in range(NIT)]
            XNT = [p_sb.tile([128, R], BF, tag=f"xnt{d}", name=f"xnt{d}")
                   for d in range(KD)]
            H1T = [p_sb.tile([128, R], BF, tag=f"h1t{f}", name=f"h1t{f}")
                   for f in range(KF)]
            XO = [p_sb.tile([128, D], F32, tag=f"xo{i}", name=f"xo{i}")
                  for i in range(NIT)]

            # ---- DMA loads -------------------------------------------
            # sync HWDGE: the P -> scores critical path, then wv, fw2
            for d in range(KD):
                nc.sync.dma_start(MM[d][:], m[128 * d:128 * (d + 1), :])
                nc.sync.dma_start(XTR[d][:], xtr[128 * d:128 * (d + 1), :])
            for e in range(KD):
                nc.sync.dma_start(XT[e][:], xT[128 * e:128 * (e + 1), :])
            for e in range(KD):
                nc.sync.dma_start(WVS[e][:], wv[128 * e:128 * (e + 1), :])
            for f in range(KF):
                nc.sync.dma_start(FW2[f][:], fw2[128 * f:128 * (f + 1), :])

            # scalar HWDGE: bias + attention row data, then fw1
            for j in range(NJT):
                nc.scalar.dma_start(BIA[j][:],
                                    biasT[128 * j:128 * (j + 1), :])
            for j in range(NJT):
                nc.scalar.dma_start(XROW[j][:],
                                    xrow[128 * j:128 * (j + 1), :])
            for i in range(NIT):
                nc.scalar.dma_start(XR[i][:], xr[128 * i:128 * (i + 1), :])
            for d in range(KD):
                nc.scalar.dma_start(FW1[d][:], fw1[128 * d:128 * (d + 1), :])

            # gpsimd SWDGE: small constants
            iddt = p_const.tile([128, 128], BF, tag="idd", name="idd")
            nc.gpsimd.dma_start(iddt[:], idd[:])
            fb2t = p_const.tile([1, D], BF, tag="fb2", name="fb2")
            nc.gpsimd.dma_start(fb2t[:], fb2[:])
            fb1tt = p_const.tile([128, KF], F32, tag="fb1t", name="fb1t")
            nc.gpsimd.dma_start(fb1tt[:], fb1t[:])
            lnp = {}
            for nm, tsr in (("ln1g", ln1g), ("ln1b", ln1b),
                            ("ln2g", ln2g), ("ln2b", ln2b)):
                row = p_const.tile([1, D], F32, tag=nm + "_r")
                nc.gpsimd.dma_start(row[:], tsr[:])
                bc = p_const.tile([128, D], F32, tag=nm + "_b")
                nc.gpsimd.partition_broadcast(bc[:], row[:])
                lnp[nm] = bc

            # ---- P = M^T x_r^T, scaled 1/sqrt(D)  [D, R] --------------
            for e in range(KD):
                ps = p_sc.tile([128, R], F32, tag="sc", name="sc")
                for d in range(KD):
                    mm(ps[:], MM[d][:, 128 * e:128 * (e + 1)], XTR[d][:],
                       d == 0, d == KD - 1)
                nc.scalar.activation(P[e][:], ps[:], AF.Copy,
                                     scale=inv_scale)

            # ---- scoresT[j, i] = bias^T + x @ P; exp ------------------
            for j in range(NJT):
                ps = p_sc.tile([128, R], F32, tag="sc", name="sc")
                mm(ps[:], iddt[:], BIA[j][:], True, False)
                for e in range(KD):
                    mm(ps[:], XT[e][:, 128 * j:128 * (j + 1)], P[e][:],
                       False, e == KD - 1)
                nc.scalar.activation(EE[j][:], ps[:], AF.Exp)

            # ---- AX[i, :] = sum_j ee[j, i] * x[j, :] ------------------
            for it in range(NIT):
                ps = p_mm.tile([128, D], F32, tag="mm", name="mm")
                for j in range(NJT):
                    mm(ps[:], EE[j][:, 128 * it:128 * (it + 1)], XROW[j][:],
                       j == 0, j == NJT - 1)
                nc.vector.tensor_copy(AXN[it][:], ps[:])

            # ---- Z[i] = sum_j ee[j, i]; rz = 1/Z ----------------------
            for it in range(NIT):
                zs = p_tp.tile([128, 1], F32, tag="tp", name="tp")
                for j in range(NJT):
                    mm(zs[:], EE[j][:, 128 * it:128 * (it + 1)], ones_c[:],
                       j == 0, j == NJT - 1)
                nc.vector.reciprocal(RZC[it][:], zs[:])

            # ---- transpose AX -> AXT ----------------------------------
            for it in range(NIT):
                for e in range(KD):
                    tp = p_tp.tile([128, 128], BF, tag="tp", name="tp")
                    nc.tensor.transpose(
                        tp[:], AXN[it][:, 128 * e:128 * (e + 1)], iddt[:])
                    nc.vector.tensor_copy(
                        AXT[e][:, 128 * it:128 * (it + 1)], tp[:])

            # ---- attn_out = AX @ Wv; z1 = rz*attn_out + x; LN1 --------
            for it in range(NIT):
                ps = p_mm.tile([128, D], F32, tag="mm", name="mm")
                for e in range(KD):
                    mm(ps[:], AXT[e][:, 128 * it:128 * (it + 1)], WVS[e][:],
                       e == 0, e == KD - 1)
                z1 = p_tmp.tile([128, D], F32, tag="z1", name="z1")
                nc.vector.scalar_tensor_tensor(
                    z1[:], ps[:], RZC[it][:], XR[it][:], ALU.mult, ALU.add)
                layernorm(XN1[it], z1, lnp["ln1g"], lnp["ln1b"], p_tmp)

            # ---- transpose XN1 -> XNT for the FFN ---------------------
            for it in range(NIT):
                for d in range(KD):
                    tp = p_tp.tile([128, 128], BF, tag="tp", name="tp")
                    nc.tensor.transpose(
                        tp[:], XN1[it][:, 128 * d:128 * (d + 1)], iddt[:])
                    nc.vector.tensor_copy(
                        XNT[d][:, 128 * it:128 * (it + 1)], tp[:])

            # ---- FFN --------------------------------------------------
            for f in range(KF):
                ps = p_sc.tile([128, R], F32, tag="sc", name="sc")
                for d in range(KD):
                    mm(ps[:], FW1[d][:, 128 * f:128 * (f + 1)], XNT[d][:],
                       d == 0, d == KD - 1)
                nc.scalar.activation(H1T[f][:], ps[:], AF.Relu,
                                     bias=fb1tt[:, f:f + 1])

            for it in range(NIT):
                ps = p_mm.tile([128, D], F32, tag="mm", name="mm")
                nc.tensor.matmul(ps[:], ones_r[:], fb2t[:],
                                 start=True, stop=False)
                for f in range(KF):
                    mm(ps[:], H1T[f][:, 128 * it:128 * (it + 1)], FW2[f][:],
                       False, f == KF - 1)
                z2 = p_tmp.tile([128, D], F32, tag="z2", name="z2")
                nc.vector.tensor_tensor(z2[:], ps[:], XN1[it][:], ALU.add)
                layernorm(XO[it], z2, lnp["ln2g"], lnp["ln2b"], p_tmp)
                nc.sync.dma_start(xout[128 * it:128 * (it + 1), :],
                                  XO[it][:])

    nc.compile()
    return nc


def _get_program():
    global _prog
    if _prog is None:
        _prog = _build_program()
    return _prog


# ----------------------------------------------------------------------------
# host glue
# ----------------------------------------------------------------------------

_exec = None        # cached (jitted_fn, in_names, out_names, out_avals, mesh)


def _get_exec(nc):
    """Build the PJRT executable once (run_bass_via_pjrt rebuilds its jit on
    every call, costing seconds of retrace; this is the same lowering with
    the jit cached)."""
    global _exec
    if _exec is not None:
        return _exec
    import jax
    import numpy as np_
    from jax.sharding import Mesh, PartitionSpec
    from jax.experimental.shard_map import shard_map
    import concourse.mybir as mybir
    from concourse.bass2jax import (_bass_exec_p, install_neuronx_cc_hook,
                                    partition_id_tensor)

    install_neuronx_cc_hook()
    partition_name = (nc.partition_id_tensor.name
                      if nc.partition_id_tensor else None)
    in_names, out_names, out_avals = [], [], []
    for alloc in nc.m.functions[0].allocations:
        if not isinstance(alloc, mybir.MemoryLocationSet):
            continue
        name = alloc.memorylocations[0].name
        if alloc.kind == "ExternalInput":
            if name != partition_name:
                in_names.append(name)
        elif alloc.kind == "ExternalOutput":
            out_names.append(name)
            out_avals.append(jax.core.ShapedArray(
                tuple(alloc.tensor_shape), mybir.dt.np(alloc.dtype)))
    n_params = len(in_names)
    n_outs = len(out_names)
    all_names = in_names + out_names
    if partition_name is not None:
        all_names.append(partition_name)
    donate = tuple(range(n_params, n_params + n_outs))

    def _body(*args):
        operands = list(args)
        if partition_name is not None:
            operands.append(partition_id_tensor())
        return tuple(_bass_exec_p.bind(
            *operands,
            out_avals=tuple(out_avals),
            in_names=tuple(all_names),
            out_names=tuple(out_names),
            lowering_input_output_aliases=(),
            sim_require_finite=True,
            sim_require_nnan=True,
            nc=nc,
        ))

    devices = jax.devices()[:NCORES]
    mesh = Mesh(np_.asarray(devices), ("core",))
    core_spec = PartitionSpec("core")
    repl_spec = PartitionSpec()
    in_specs = tuple(core_spec if n in _VARYING else repl_spec
                     for n in in_names) + (core_spec,) * n_outs
    fn = jax.jit(
        shard_map(_body, mesh=mesh,
                  in_specs=in_specs,
                  out_specs=(core_spec,) * n_outs,
                  check_rep=False),
        donate_argnums=donate, keep_unused=True)
    _exec = (fn, in_names, out_names, out_avals, mesh)
    return _exec


_VARYING = {"xT", "xtr", "xrow", "xr", "biasT"}
_repl_cache = {}


def _repl_device_put(name, arr, mesh):
    """Upload a replicated input once; reuse device array on same content."""
    import hashlib
    import jax
    from jax.sharding import NamedSharding, PartitionSpec
    key = (name, arr.shape, hashlib.blake2b(arr.tobytes(),
                                            digest_size=16).digest())
    hit = _repl_cache.get(key)
    if hit is not None:
        return hit
    dev = jax.device_put(arr, NamedSharding(mesh, PartitionSpec()))
    _repl_cache[key] = dev
    if len(_repl_cache) > 64:
        _repl_cache.pop(next(iter(_repl_cache)))
    return dev


def _run_fast(nc, in_maps):
    fn, in_names, out_names, out_avals, mesh = _get_exec(nc)
    args = []
    for n in in_names:
        if n in _VARYING:
            args.append(np.concatenate([m[n] for m in in_maps], axis=0))
        else:
            args.append(_repl_device_put(n, in_maps[0][n], mesh))
    zeros = [np.zeros((NCORES * a.shape[0], *a.shape[1:]), a.dtype)
             for a in out_avals]
    outs = fn(*args, *zeros)
    res = []
    for c in range(NCORES):
        res.append({n: np.asarray(outs[i]).reshape(
            NCORES, *out_avals[i].shape)[c]
            for i, n in enumerate(out_names)})
    return res


def _launch(nc, x, bias_rows, inputs, layer, trace=False):
    """One transformer layer across 8 cores. Returns (x_next, None, results)."""
    from concourse.bass_utils import run_bass_kernel_spmd

    idd = np.eye(128, dtype=BF16)
    fb1t = np.ascontiguousarray(
        inputs["ffn_b1"][layer].reshape(KF, 128).T.astype(np.float32))
    M = (inputs["Wq"][layer] @ inputs["Wk"][layer].T).astype(np.float32)

    xT_b = [np.ascontiguousarray(x[b].T.astype(BF16)) for b in range(B)]
    xrow_b = [np.ascontiguousarray(x[b].astype(BF16)) for b in range(B)]

    in_maps = []
    for core in range(NCORES):
        b, q = divmod(core, QB)
        r0 = q * R
        mp = {
            "xT": xT_b[b],
            "xtr": np.ascontiguousarray(xT_b[b][:, r0:r0 + R]),
            "xrow": xrow_b[b],
            "xr": np.ascontiguousarray(x[b][r0:r0 + R]).astype(np.float32),
            "m": M.astype(BF16),
            "wv": inputs["Wv"][layer].astype(BF16),
            "biasT": np.ascontiguousarray(
                bias_rows[b][r0:r0 + R].T).astype(BF16),
            "fw1": inputs["ffn_w1"][layer].astype(BF16),
            "fb1t": fb1t,
            "fw2": inputs["ffn_w2"][layer].astype(BF16),
            "fb2": inputs["ffn_b2"][layer].reshape(1, D).astype(BF16),
            "ln1g": inputs["ln1_g"][layer].reshape(1, D).astype(np.float32),
            "ln1b": inputs["ln1_b"][layer].reshape(1, D).astype(np.float32),
            "ln2g": inputs["ln2_g"][layer].reshape(1, D).astype(np.float32),
            "ln2b": inputs["ln2_b"][layer].reshape(1, D).astype(np.float32),
            "idd": idd,
        }
        in_maps.append({k: np.ascontiguousarray(v) for k, v in mp.items()})

    if trace:
        res = run_bass_kernel_spmd(nc, in_maps, list(range(NCORES)),
                                   trace=True)
        outs = res.results
    else:
        res = None
        outs = _run_fast(nc, in_maps)
    x_next = np.empty((B, S, D), np.float32)
    for core in range(NCORES):
        b, q = divmod(core, QB)
        x_next[b, q * R:(q + 1) * R] = outs[core]["xout"]
    return x_next, None, res


def _host_head(x, inputs):
    """Final LN + mean pool + fc on host."""
    g = inputs["lnf_g"].astype(np.float32)
    bb = inputs["lnf_b"].astype(np.float32)
    mu = x.mean(-1, keepdims=True)
    var = ((x - mu) ** 2).mean(-1, keepdims=True)
    xn = (x - mu) / np.sqrt(var + EPS_LN) * g + bb
    pooled = xn.mean(axis=1)
    return pooled @ inputs["fc_w"].astype(np.float32) \
        + inputs["fc_b"].astype(np.float32)[None, :]


def kernel(**inputs):
    inputs = {k: np.asarray(v) for k, v in inputs.items()}
    nc = _get_program()
    x = np.asarray(inputs["x"], np.float32)
    for layer in range(L):
        bias_rows = _host_bias_rows(inputs, layer)
        x, _, _ = _launch(nc, x, bias_rows, inputs, layer)
    out = _host_head(x, inputs)
    return out.astype(np.float32)


# revision 3
# speedup vs baseline: 2.2161x; 1.2412x over previous
"""Trainium2 Bass kernel for nn_CombinedNN_65635690217686.

2-layer transformer with pairwise-geometry score biases.
Sharding: 8 cores = 2 batches x 4 query-row-blocks (256 rows each).
One Bass program (a single transformer layer), launched twice (layer 0,
layer 1); host gathers/reshards x between launches.

Key restructurings vs the straightforward per-layer program:
  * scores = x (Wq Wk^T) x^T / sqrt(D): M = Wq@Wk^T is precomputed on
    host, so the device does P = M^T x_r^T ([D,R]) then scoresT = x P
    -- no K or Q projection at all.
  * attn_out = (A @ x) @ Wv (associativity) -- no V projection.
  * scores are built TRANSPOSED ([j, i]) so the softmax denominator
    comes from ones-matmuls and A@x needs no transpose of the attention
    matrix; softmax skips the max-subtraction (scores are O(1)).
  * layernorm means come from the ACT engine's free accumulator during
    the PSUM evacuation (no DVE reduction); LN1's beta is folded into
    the FFN transpose evacuation + FFN2 PSUM preload; LN2's beta is
    applied on host.  Residual adds ride through PSUM via identity
    matmuls.
  * every DRAM tensor is packed so each DMA is a single instruction
    with wide (>=2KB) contiguous rows; bulk loads are issued on the
    queues so they never head-of-line block ACT compute.
  * the classifier head (final LN + mean-pool + fc) runs on host.
  * all matmul operands are bf16 (same PE rate as f32r, half the HBM
    traffic); accumulation stays fp32 in PSUM.

The O(S^2) pairwise-bias MLPs: bias(i,j) depends only on
rel = coords_j - coords_i.  setup_inputs() places coords on an exact
32x32 grid, so rel takes only 63x63 distinct values; the host evaluates
the three tiny MLPs on those 3969 classes and expands to per-row bias
tables that the device consumes directly.  If coords are NOT the grid
(defensive fallback), the host evaluates the exact MLPs on all S^2
pairs instead.
"""

import math
import sys

import numpy as np
import ml_dtypes

sys.path.insert(0, "/opt/trn_rl_repo")

BF16 = ml_dtypes.bfloat16

L, B, S, D, H, F, C = 2, 2, 1024, 512, 32, 2048, 1000
EPS_LN = 1e-5
NCORES = 8
QB = 4              # query blocks per batch
R = S // QB         # 256 rows per core
G = 32              # coord grid side
NDIFF = 2 * G - 1   # 63 difference classes per axis

KD = D // 128       # 4 chunks over D
KF = F // 128       # 16 chunks over F
NIT = R // 128      # 2 query i-tiles per core
NJT = S // 128      # 8 j row-chunks

_prog = None        # cached Bass program


# ----------------------------------------------------------------------------
# host-side pairwise-bias evaluation
# ----------------------------------------------------------------------------

def _grid_coords_np():
    g = math.ceil(math.sqrt(S))
    xs = np.linspace(0.0, 1.0, g, dtype=np.float64).astype(np.float32)
    gx, gy = np.meshgrid(xs, xs, indexing="ij")
    pts = np.stack([gx.ravel(), gy.ravel()], axis=1)
    reps = math.ceil(S / (g * g))
    pts = np.tile(pts, (reps, 1))[:S]
    return np.broadcast_to(pts[None], (B, S, 2)).astype(np.float32)


def _pair_bias_from_rel(dx, dy, rot_w1, rot_b1, rot_w2,
                        trans_w1, trans_b1, trans_w2,
                        refl_w1, refl_b1, refl_w2):
    """Exact reference pairwise bias (minus the softmax-invariant b2 consts)."""
    dx = dx.astype(np.float32)
    dy = dy.astype(np.float32)
    dist = np.sqrt(dx * dx + dy * dy + np.float32(1e-8))
    theta = np.arctan2(dy, dx)
    rot_in = np.stack([dist, np.sin(theta), np.cos(theta)], axis=-1)
    trans_in = np.stack([dx, dy], axis=-1)
    refl_in = np.concatenate([trans_in, -trans_in], axis=-1)

    def mlp(inp, w1, b1, w2):
        h = np.maximum(inp @ w1 + b1, 0.0)
        return h @ w2

    out = (mlp(rot_in, rot_w1, rot_b1, rot_w2)
           + mlp(trans_in, trans_w1, trans_b1, trans_w2)
           + mlp(refl_in, refl_w1, refl_b1, refl_w2))
    return out.astype(np.float32)


def _expand_idx():
    """idx[i, j] -> difference-class index into the flat 63x63 table."""
    i = np.arange(S)
    ai, bi = i // G, i % G
    da = ai[None, :] - ai[:, None] + (G - 1)
    db = bi[None, :] - bi[:, None] + (G - 1)
    return (da * NDIFF + db).astype(np.int32)


_IDX = None


def _host_bias_rows(inputs, layer):
    """Full bias rows [B, S, S] float32 for one layer."""
    global _IDX
    args = (inputs["rot_w1"][layer], inputs["rot_b1"][layer],
            inputs["rot_w2"][layer],
            inputs["trans_w1"][layer], inputs["trans_b1"][layer],
            inputs["trans_w2"][layer],
            inputs["refl_w1"][layer], inputs["refl_b1"][layer],
            inputs["refl_w2"][layer])
    coords = np.asarray(inputs["coords"], np.float32)
    if np.array_equal(coords, _grid_coords_np()):
        d = (np.arange(NDIFF, dtype=np.float64) - (G - 1)) / (G - 1)
        dxg, dyg = np.meshgrid(d, d, indexing="ij")
        tab = _pair_bias_from_rel(dxg, dyg, *args).ravel()
        if _IDX is None:
            _IDX = _expand_idx()
        full = tab[_IDX]
        return np.broadcast_to(full[None], (B, S, S))
    out = np.empty((B, S, S), np.float32)
    for b in range(B):
        cb = coords[b]
        dx = cb[None, :, 0] - cb[:, None, 0]
        dy = cb[None, :, 1] - cb[:, None, 1]
        out[b] = _pair_bias_from_rel(dx, dy, *args)
    return out


def _pack(arr, k, n):
    """[k*128, n] -> [128, k*n]: chunk c's rows land at cols [c*n,(c+1)*n)."""
    return np.ascontiguousarray(
        arr.reshape(k, 128, n).transpose(1, 0, 2).reshape(128, k * n))


# ----------------------------------------------------------------------------
# device program: one transformer layer for 256 query rows of one batch
# ----------------------------------------------------------------------------

def _build_program():
    import concourse.mybir as mybir
    import concourse.tile as tile
    from concourse import bacc

    F32 = mybir.dt.float32
    BF = mybir.dt.bfloat16
    AF = mybir.ActivationFunctionType
    ALU = mybir.AluOpType

    nc = bacc.Bacc()

    def din(name, shape, dt=BF):
        return nc.dram_tensor(name, shape, dt, kind="ExternalInput")

    # packed layouts: [128, chunks * width]
    xt2 = din("xt2", [128, KD * S])       # x^T chunks (d-major)
    xtr2 = din("xtr2", [128, KD * R])     # x^T, this core's query columns
    xrow2 = din("xrow2", [128, NJT * D])  # x row chunks
    xr2 = din("xr2", [128, NIT * D], F32)  # residual rows (fp32)
    sxr = din("sxr", [128, NIT], F32)     # row-sums of xr (for LN1 mean)
    m2 = din("m2", [128, KD * D])         # M = Wq @ Wk^T, d-chunked
    wv2 = din("wv2", [128, KD * D])
    bias2 = din("bias2", [128, NJT * R])  # pairwise bias^T chunks
    fw1x = din("fw1x", [128, KD * F])
    fb1t = din("fb1t", [128, KF], F32)
    fw2x = din("fw2x", [128, KF * D])
    fb2x = din("fb2x", [1, D])            # ffn_b2 + ln1_b (host-folded)
    ln1bt = din("ln1bt", [128, KD], F32)  # ln1_b chunked per-partition
    ln1g = din("ln1g", [1, D], F32)
    ln2g = din("ln2g", [1, D], F32)
    idd = din("idd", [128, 128])          # bf16 identity

    xout = nc.dram_tensor("xout", [R, D], F32, kind="ExternalOutput")

    inv_scale = 1.0 / math.sqrt(D)

    def mm(out, lhsT, rhs, start, stop):
        nc.tensor.matmul(out, lhsT, rhs, start=start, stop=stop)

    with tile.TileContext(nc) as tc:
        from contextlib import ExitStack
        es = ExitStack()
        with es:
            p_const = es.enter_context(tc.tile_pool(name="const", bufs=1))
            # PSUM banks: mm 3 + sc 3 + tp 2 = 8
            p_mm = es.enter_context(
                tc.tile_pool(name="pmm", bufs=3, space="PSUM"))
            p_sc = es.enter_context(
                tc.tile_pool(name="psc", bufs=3, space="PSUM"))
            p_tp = es.enter_context(
                tc.tile_pool(name="ptp", bufs=2, space="PSUM"))

            p_sb = es.enter_context(tc.tile_pool(name="sb", bufs=1))
            p_tmp = es.enter_context(tc.tile_pool(name="tmp", bufs=2))

            ones_c = p_const.tile([128, 1], BF, tag="ones_c", name="ones_c")
            nc.vector.memset(ones_c[:], 1.0)
            ones_r = p_const.tile([1, 128], BF, tag="ones_r", name="ones_r")
            nc.vector.memset(ones_r[:], 1.0)
            ones_1 = p_const.tile([1, 1], F32, tag="ones_1", name="ones_1")
            nc.vector.memset(ones_1[:], 1.0)
            eps_t = p_const.tile([128, 1], F32, tag="eps", name="eps")
            nc.vector.memset(eps_t[:], EPS_LN)

            # ---- persistent SBUF tiles (packed, sliced per chunk) -----
            XT = p_sb.tile([128, KD * S], BF, tag="xt", name="xt")
            XTR = p_sb.tile([128, KD * R], BF, tag="xtr", name="xtr")
            XRW = p_sb.tile([128, NJT * D], BF, tag="xrw", name="xrw")
            XR = p_sb.tile([128, NIT * D], F32, tag="xr", name="xr")
            SXR = p_sb.tile([128, NIT], F32, tag="sxr", name="sxr")
            MM2 = p_sb.tile([128, KD * D], BF, tag="m2", name="m2")
            WV = p_sb.tile([128, KD * D], BF, tag="wv", name="wv")
            BIA = p_sb.tile([128, NJT * R], BF, tag="bia", name="bia")
            FW1 = p_sb.tile([128, KD * F], BF, tag="fw1", name="fw1")
            FW2 = p_sb.tile([128, KF * D], BF, tag="fw2", name="fw2")
            P = p_sb.tile([128, KD * R], BF, tag="p", name="p")
            EE = p_sb.tile([128, NJT * R], BF, tag="ee", name="ee")
            AXN = [p_sb.tile([128, D], BF, tag=f"axn{i}", name=f"axn{i}")
                   for i in range(NIT)]
            AXT = p_sb.tile([128, KD * R], BF, tag="axt", name="axt")
            RZR = p_sb.tile([1, R], F32, tag="rzr", name="rzr")
            RZC = [p_sb.tile([128, 1], F32, tag=f"rz{i}", name=f"rz{i}")
                   for i in range(NIT)]
            XN1 = [p_sb.tile([128, D], BF, tag=f"xn1_{i}", name=f"xn1_{i}")
                   for i in range(NIT)]
            XNT = p_sb.tile([128, KD * R], BF, tag="xnt", name="xnt")
            H1T = p_sb.tile([128, KF * R], BF, tag="h1t", name="h1t")
            XO = [p_sb.tile([128, D], F32, tag=f"xo{i}", name=f"xo{i}")
                  for i in range(NIT)]

            def xts(e, j0, w):      # x^T chunk e, col slice
                return XT[:, S * e + j0: S * e + j0 + w]

            def sl(tile_, c, w, o0=0, ow=None):
                base = c * w + o0
                return tile_[:, base: base + (ow if ow is not None else w)]

            # ---- DMA: sync queue carries the early critical path ------
            nc.sync.dma_start(MM2[:], m2[:])
            nc.sync.dma_start(XTR[:], xtr2[:])
            nc.sync.dma_start(XT[:], xt2[:])
            nc.sync.dma_start(XRW[:], xrow2[:])
            nc.sync.dma_start(WV[:], wv2[:])

            # scalar queue: only the bias before ACT's compute section
            nc.scalar.dma_start(BIA[:], bias2[:])

            # gpsimd SWDGE: small constants
            iddt = p_const.tile([128, 128], BF, tag="idd", name="idd")
            nc.gpsimd.dma_start(iddt[:], idd[:])
            fb2t = p_const.tile([1, D], BF, tag="fb2", name="fb2")
            nc.gpsimd.dma_start(fb2t[:], fb2x[:])
            fb1tt = p_const.tile([128, KF], F32, tag="fb1t", name="fb1t")
            nc.gpsimd.dma_start(fb1tt[:], fb1t[:])
            l1bt = p_const.tile([128, KD], F32, tag="l1bt", name="l1bt")
            nc.gpsimd.dma_start(l1bt[:], ln1bt[:])
            nc.gpsimd.dma_start(SXR[:], sxr[:])
            lnp = {}
            for nm, tsr in (("ln1g", ln1g), ("ln2g", ln2g)):
                row = p_const.tile([1, D], F32, tag=nm + "_r")
                nc.gpsimd.dma_start(row[:], tsr[:])
                bc = p_const.tile([128, D], F32, tag=nm + "_b")
                nc.gpsimd.partition_broadcast(bc[:], row[:])
                lnp[nm] = bc

            # ---- P = M^T x_r^T, scaled 1/sqrt(D)  [D, R] --------------
            for e in range(KD):
                ps = p_sc.tile([128, R], F32, tag="sc", name="sc")
                for d in range(KD):
                    mm(ps[:], MM2[:, D * d + 128 * e: D * d + 128 * (e + 1)],
                       sl(XTR, d, R), d == 0, d == KD - 1)
                nc.scalar.activation(sl(P, e, R), ps[:], AF.Copy,
                                     scale=inv_scale)

            # ---- scoresT[j, i] = x @ P + bias^T; exp ------------------
            for j in range(NJT):
                ps = p_sc.tile([128, R], F32, tag="sc", name="sc")
                for e in range(KD):
                    mm(ps[:], xts(e, 128 * j, 128), sl(P, e, R),
                       e == 0, e == KD - 1)
                nc.vector.tensor_tensor(ps[:], ps[:], sl(BIA, j, R), ALU.add)
                nc.scalar.activation(sl(EE, j, R), ps[:], AF.Exp)

            # late bulk loads, issued from the scalar queue AFTER the
            # exp instructions so they cannot head-of-line block ACT
            nc.scalar.dma_start(XR[:], xr2[:])
            nc.scalar.dma_start(FW1[:], fw1x[:])
            nc.scalar.dma_start(FW2[:], fw2x[:])

            # ---- AX[i, :] = sum_j ee[j, i] * x[j, :] ------------------
            for it in range(NIT):
                ps = p_mm.tile([128, D], F32, tag="mm", name="mm")
                for j in range(NJT):
                    mm(ps[:], sl(EE, j, R, 128 * it, 128), sl(XRW, j, D),
                       j == 0, j == NJT - 1)
                nc.vector.tensor_copy(AXN[it][:], ps[:])

            # ---- Z row = ones^T @ EE; rz = 1/Z; spread to columns -----
            zs = p_tp.tile([1, R], F32, tag="tp", name="tp")
            for j in range(NJT):
                mm(zs[:], ones_c[:], sl(EE, j, R), j == 0, j == NJT - 1)
            nc.vector.reciprocal(RZR[:], zs[:])
            for it in range(NIT):
                cs = p_tp.tile([128, 1], F32, tag="tp", name="tp")
                mm(cs[:], RZR[:, 128 * it:128 * (it + 1)], ones_1[:],
                   True, True)
                nc.vector.tensor_copy(RZC[it][:], cs[:])

            # ---- transpose AX -> AXT; attn_out = AX @ Wv; LN1 ---------
            for it in range(NIT):
                for e in range(KD):
                    tp = p_tp.tile([128, 128], BF, tag="tp", name="tp")
                    nc.tensor.transpose(
                        tp[:], AXN[it][:, 128 * e:128 * (e + 1)], iddt[:])
                    nc.vector.tensor_copy(
                        sl(AXT, e, R, 128 * it, 128), tp[:])
                ps = p_mm.tile([128, D], F32, tag="mm", name="mm")
                for e in range(KD):
                    mm(ps[:], sl(AXT, e, R, 128 * it, 128), sl(WV, e, D),
                       e == 0, e == KD - 1)
                # t = rz * attn_out (ACT evac), s1 = row-sum(t)
                t = p_tmp.tile([128, D], F32, tag="t", name="t")
                s1 = p_tmp.tile([128, 1], F32, tag="s1", name="s1")
                nc.scalar.activation(t[:], ps[:], AF.Copy,
                                     scale=RZC[it][:], accum_out=s1[:])
                # mu_neg = -(s1 + sxr)/D
                mun = p_tmp.tile([128, 1], F32, tag="mun", name="mun")
                nc.vector.tensor_tensor(mun[:], s1[:],
                                        SXR[:, it:it + 1], ALU.add)
                nc.vector.tensor_scalar_mul(mun[:], mun[:], -1.0 / D)
                # zc = (t + mu_neg) + xr
                zc = p_tmp.tile([128, D], F32, tag="zc", name="zc")
                nc.vector.scalar_tensor_tensor(
                    zc[:], t[:], mun[:], sl(XR, it, D), ALU.add, ALU.add)
                # var, rstd
                s2 = p_tmp.tile([128, 1], F32, tag="s2", name="s2")
                nc.scalar.activation(t[:], zc[:], AF.Square, accum_out=s2[:])
                std = p_tmp.tile([128, 1], F32, tag="std", name="std")
                nc.scalar.activation(std[:], s2[:], AF.Sqrt,
                                     scale=1.0 / D, bias=eps_t[:])
                rstd = p_tmp.tile([128, 1], F32, tag="rstd", name="rstd")
                nc.vector.reciprocal(rstd[:], std[:])
                # xn1 = zc * rstd * g   (beta folded downstream)
                nc.vector.scalar_tensor_tensor(
                    XN1[it][:], zc[:], rstd[:], lnp["ln1g"][:],
                    ALU.mult, ALU.mult)

            # ---- transpose XN1 -> XNT (+ln1_b on evacuation) ----------
            for it in range(NIT):
                for d in range(KD):
                    tp = p_tp.tile([128, 128], BF, tag="tp", name="tp")
                    nc.tensor.transpose(
                        tp[:], XN1[it][:, 128 * d:128 * (d + 1)], iddt[:])
                    nc.scalar.activation(
                        sl(XNT, d, R, 128 * it, 128), tp[:], AF.Identity,
                        bias=l1bt[:, d:d + 1])

            # ---- FFN --------------------------------------------------
            for f in range(KF):
                ps = p_sc.tile([128, R], F32, tag="sc", name="sc")
                for d in range(KD):
                    mm(ps[:], FW1[:, F * d + 128 * f: F * d + 128 * (f + 1)],
                       sl(XNT, d, R), d == 0, d == KD - 1)
                nc.scalar.activation(sl(H1T, f, R), ps[:], AF.Relu,
                                     bias=fb1tt[:, f:f + 1])

            for it in range(NIT):
                ps = p_mm.tile([128, D], F32, tag="mm", name="mm")
                # preload (ffn_b2 + ln1_b) broadcast + residual xn1
                nc.tensor.matmul(ps[:], ones_r[:], fb2t[:],
                                 start=True, stop=False)
                nc.tensor.matmul(ps[:], iddt[:], XN1[it][:],
                                 start=False, stop=False)
                for f in range(KF):
                    mm(ps[:], sl(H1T, f, R, 128 * it, 128), sl(FW2, f, D),
                       False, f == KF - 1)
                # z2 sits entirely in PSUM; evac + mean via ACT accum
                t = p_tmp.tile([128, D], F32, tag="t", name="t")
                s1 = p_tmp.tile([128, 1], F32, tag="s1", name="s1")
                nc.scalar.activation(t[:], ps[:], AF.Copy, accum_out=s1[:])
                mun = p_tmp.tile([128, 1], F32, tag="mun", name="mun")
                nc.vector.tensor_scalar_mul(mun[:], s1[:], -1.0 / D)
                zc = p_tmp.tile([128, D], F32, tag="zc", name="zc")
                nc.vector.tensor_scalar_add(zc[:], t[:], mun[:])
                s2 = p_tmp.tile([128, 1], F32, tag="s2", name="s2")
                nc.scalar.activation(t[:], zc[:], AF.Square, accum_out=s2[:])
                std = p_tmp.tile([128, 1], F32, tag="std", name="std")
                nc.scalar.activation(std[:], s2[:], AF.Sqrt,
                                     scale=1.0 / D, bias=eps_t[:])
                rstd = p_tmp.tile([128, 1], F32, tag="rstd", name="rstd")
                nc.vector.reciprocal(rstd[:], std[:])
                nc.vector.scalar_tensor_tensor(
                    XO[it][:], zc[:], rstd[:], lnp["ln2g"][:],
                    ALU.mult, ALU.mult)
                nc.sync.dma_start(xout[128 * it:128 * (it + 1), :],
                                  XO[it][:])

    nc.compile()
    return nc


def _get_program():
    global _prog
    if _prog is None:
        _prog = _build_program()
    return _prog


# ----------------------------------------------------------------------------
# host glue
# ----------------------------------------------------------------------------

_exec = None        # cached (jitted_fn, in_names, out_names, out_avals, mesh)


def _get_exec(nc):
    """Build the PJRT executable once (run_bass_via_pjrt rebuilds its jit on
    every call, costing seconds of retrace; this is the same lowering with
    the jit cached)."""
    global _exec
    if _exec is not None:
        return _exec
    import jax
    import numpy as np_
    from jax.sharding import Mesh, PartitionSpec
    from jax.experimental.shard_map import shard_map
    import concourse.mybir as mybir
    from concourse.bass2jax import (_bass_exec_p, install_neuronx_cc_hook,
                                    partition_id_tensor)

    install_neuronx_cc_hook()
    partition_name = (nc.partition_id_tensor.name
                      if nc.partition_id_tensor else None)
    in_names, out_names, out_avals = [], [], []
    for alloc in nc.m.functions[0].allocations:
        if not isinstance(alloc, mybir.MemoryLocationSet):
            continue
        name = alloc.memorylocations[0].name
        if alloc.kind == "ExternalInput":
            if name != partition_name:
                in_names.append(name)
        elif alloc.kind == "ExternalOutput":
            out_names.append(name)
            out_avals.append(jax.core.ShapedArray(
                tuple(alloc.tensor_shape), mybir.dt.np(alloc.dtype)))
    n_params = len(in_names)
    n_outs = len(out_names)
    all_names = in_names + out_names
    if partition_name is not None:
        all_names.append(partition_name)
    donate = tuple(range(n_params, n_params + n_outs))

    def _body(*args):
        operands = list(args)
        if partition_name is not None:
            operands.append(partition_id_tensor())
        return tuple(_bass_exec_p.bind(
            *operands,
            out_avals=tuple(out_avals),
            in_names=tuple(all_names),
            out_names=tuple(out_names),
            lowering_input_output_aliases=(),
            sim_require_finite=True,
            sim_require_nnan=True,
            nc=nc,
        ))

    devices = jax.devices()[:NCORES]
    mesh = Mesh(np_.asarray(devices), ("core",))
    core_spec = PartitionSpec("core")
    repl_spec = PartitionSpec()
    in_specs = tuple(core_spec if n in _VARYING else repl_spec
                     for n in in_names) + (core_spec,) * n_outs
    fn = jax.jit(
        shard_map(_body, mesh=mesh,
                  in_specs=in_specs,
                  out_specs=(core_spec,) * n_outs,
                  check_rep=False),
        donate_argnums=donate, keep_unused=True)
    _exec = (fn, in_names, out_names, out_avals, mesh)
    return _exec


_VARYING = {"xt2", "xtr2", "xrow2", "xr2", "sxr", "bias2"}
_repl_cache = {}


def _repl_device_put(name, arr, mesh):
    """Upload a replicated input once; reuse device array on same content."""
    import hashlib
    import jax
    from jax.sharding import NamedSharding, PartitionSpec
    key = (name, arr.shape, hashlib.blake2b(arr.tobytes(),
                                            digest_size=16).digest())
    hit = _repl_cache.get(key)
    if hit is not None:
        return hit
    dev = jax.device_put(arr, NamedSharding(mesh, PartitionSpec()))
    _repl_cache[key] = dev
    if len(_repl_cache) > 64:
        _repl_cache.pop(next(iter(_repl_cache)))
    return dev


def _run_fast(nc, in_maps):
    fn, in_names, out_names, out_avals, mesh = _get_exec(nc)
    args = []
    for n in in_names:
        if n in _VARYING:
            args.append(np.concatenate([m[n] for m in in_maps], axis=0))
        else:
            args.append(_repl_device_put(n, in_maps[0][n], mesh))
    zeros = [np.zeros((NCORES * a.shape[0], *a.shape[1:]), a.dtype)
             for a in out_avals]
    outs = fn(*args, *zeros)
    res = []
    for c in range(NCORES):
        res.append({n: np.asarray(outs[i]).reshape(
            NCORES, *out_avals[i].shape)[c]
            for i, n in enumerate(out_names)})
    return res


def _launch(nc, x, bias_rows, inputs, layer, trace=False):
    """One transformer layer across 8 cores. Returns (x_next, None, results)."""
    from concourse.bass_utils import run_bass_kernel_spmd

    idd = np.eye(128, dtype=BF16)
    fb1t = np.ascontiguousarray(
        inputs["ffn_b1"][layer].reshape(KF, 128).T.astype(np.float32))
    ln1b = inputs["ln1_b"][layer].astype(np.float32)
    fb2x = (inputs["ffn_b2"][layer].astype(np.float32)
            + ln1b).reshape(1, D).astype(BF16)
    ln1bt = np.ascontiguousarray(
        ln1b.reshape(KD, 128).T.astype(np.float32))
    M = (inputs["Wq"][layer] @ inputs["Wk"][layer].T).astype(np.float32)

    m2 = _pack(M.astype(BF16), KD, D)
    wv2 = _pack(inputs["Wv"][layer].astype(BF16), KD, D)
    fw1x = _pack(inputs["ffn_w1"][layer].astype(BF16), KD, F)
    fw2x = _pack(inputs["ffn_w2"][layer].astype(BF16), KF, D)

    xT_b, xrow_b = [], []
    for b in range(B):
        xTb = np.ascontiguousarray(x[b].T.astype(BF16))
        xT_b.append(xTb)
        xrow_b.append(_pack(x[b].astype(BF16), NJT, D))

    in_maps = []
    for core in range(NCORES):
        b, q = divmod(core, QB)
        r0 = q * R
        xr = np.ascontiguousarray(x[b][r0:r0 + R]).astype(np.float32)
        mp = {
            "xt2": _pack(xT_b[b], KD, S),
            "xtr2": _pack(np.ascontiguousarray(xT_b[b][:, r0:r0 + R]),
                          KD, R),
            "xrow2": xrow_b[b],
            "xr2": _pack(xr, NIT, D),
            "sxr": np.ascontiguousarray(
                xr.sum(axis=1).reshape(NIT, 128).T.astype(np.float32)),
            "m2": m2,
            "wv2": wv2,
            "bias2": _pack(np.ascontiguousarray(
                bias_rows[b][r0:r0 + R].T).astype(BF16), NJT, R),
            "fw1x": fw1x,
            "fb1t": fb1t,
            "fw2x": fw2x,
            "fb2x": fb2x,
            "ln1bt": ln1bt,
            "ln1g": inputs["ln1_g"][layer].reshape(1, D).astype(np.float32),
            "ln2g": inputs["ln2_g"][layer].reshape(1, D).astype(np.float32),
            "idd": idd,
        }
        in_maps.append({k: np.ascontiguousarray(v) for k, v in mp.items()})

    if trace:
        res = run_bass_kernel_spmd(nc, in_maps, list(range(NCORES)),
                                   trace=True)
        outs = res.results
    else:
        res = None
        outs = _run_fast(nc, in_maps)
    ln2b = inputs["ln2_b"][layer].astype(np.float32)
    x_next = np.empty((B, S, D), np.float32)
    for core in range(NCORES):
        b, q = divmod(core, QB)
        x_next[b, q * R:(q + 1) * R] = outs[core]["xout"] + ln2b[None, :]
    return x_next, None, res


def _host_head(x, inputs):
    """Final LN + mean pool + fc on host."""
    g = inputs["lnf_g"].astype(np.float32)
    bb = inputs["lnf_b"].astype(np.float32)
    mu = x.mean(-1, keepdims=True)
    var = ((x - mu) ** 2).mean(-1, keepdims=True)
    xn = (x - mu) / np.sqrt(var + EPS_LN) * g + bb
    pooled = xn.mean(axis=1)
    return pooled @ inputs["fc_w"].astype(np.float32) \
        + inputs["fc_b"].astype(np.float32)[None, :]


def kernel(**inputs):
    inputs = {k: np.asarray(v) for k, v in inputs.items()}
    nc = _get_program()
    x = np.asarray(inputs["x"], np.float32)
    for layer in range(L):
        bias_rows = _host_bias_rows(inputs, layer)
        x, _, _ = _launch(nc, x, bias_rows, inputs, layer)
    out = _host_head(x, inputs)
    return out.astype(np.float32)


# revision 38
# speedup vs baseline: 2.3035x; 1.0395x over previous
"""Trainium2 Bass kernel for nn_CombinedNN_65635690217686.

2-layer transformer with pairwise-geometry score biases.
Sharding: 8 cores = 2 batches x 4 query-row-blocks (256 rows each).
One Bass program (a single transformer layer), launched twice (layer 0,
layer 1); host gathers/reshards x between launches.

Key restructurings vs the straightforward per-layer program:
  * scores = x (Wq Wk^T) x^T / sqrt(D): M = Wq@Wk^T is precomputed on
    host, so the device does P = M^T x_r^T ([D,R]) then scoresT = x P
    -- no K or Q projection at all.
  * attn_out = (A @ x) @ Wv (associativity) -- no V projection.
  * scores are built TRANSPOSED ([j, i]) so the softmax denominator
    comes from ones-matmuls and A@x needs no transpose of the attention
    matrix; softmax skips the max-subtraction (scores are O(1)).
  * layernorm means come from the ACT engine's free accumulator during
    the PSUM evacuation (no DVE reduction); LN1's beta is folded into
    the FFN transpose evacuation + FFN2 PSUM preload; LN2's beta is
    applied on host.  Residual adds ride through PSUM via identity
    matmuls.
  * every DRAM tensor is packed so each DMA is a single instruction
    with wide (>=2KB) contiguous rows; bulk loads are issued on the
    queues so they never head-of-line block ACT compute.
  * the classifier head (final LN + mean-pool + fc) runs on host.
  * all matmul operands are bf16 (same PE rate as f32r, half the HBM
    traffic); accumulation stays fp32 in PSUM.

The O(S^2) pairwise-bias MLPs: bias(i,j) depends only on
rel = coords_j - coords_i.  setup_inputs() places coords on an exact
32x32 grid, so rel takes only 63x63 distinct values; the host evaluates
the three tiny MLPs on those 3969 classes and expands to per-row bias
tables that the device consumes directly.  If coords are NOT the grid
(defensive fallback), the host evaluates the exact MLPs on all S^2
pairs instead.
"""

import math
import sys

import numpy as np
import ml_dtypes

sys.path.insert(0, "/opt/trn_rl_repo")

BF16 = ml_dtypes.bfloat16
F8 = ml_dtypes.float8_e4m3
SC_SCALE = 64.0     # fp8 score-path pre-scale (power of 2: exact to undo)

L, B, S, D, H, F, C = 2, 2, 1024, 512, 32, 2048, 1000
EPS_LN = 1e-5
NCORES = 8
QB = 4              # query blocks per batch
R = S // QB         # 256 rows per core
G = 32              # coord grid side
NDIFF = 2 * G - 1   # 63 difference classes per axis

KD = D // 128       # 4 chunks over D
KF = F // 128       # 16 chunks over F
NIT = R // 128      # 2 query i-tiles per core
NJT = S // 128      # 8 j row-chunks

_prog = None        # cached Bass program


# ----------------------------------------------------------------------------
# host-side pairwise-bias evaluation
# ----------------------------------------------------------------------------

def _grid_coords_np():
    g = math.ceil(math.sqrt(S))
    xs = np.linspace(0.0, 1.0, g, dtype=np.float64).astype(np.float32)
    gx, gy = np.meshgrid(xs, xs, indexing="ij")
    pts = np.stack([gx.ravel(), gy.ravel()], axis=1)
    reps = math.ceil(S / (g * g))
    pts = np.tile(pts, (reps, 1))[:S]
    return np.broadcast_to(pts[None], (B, S, 2)).astype(np.float32)


def _pair_bias_from_rel(dx, dy, rot_w1, rot_b1, rot_w2,
                        trans_w1, trans_b1, trans_w2,
                        refl_w1, refl_b1, refl_w2):
    """Exact reference pairwise bias (minus the softmax-invariant b2 consts)."""
    dx = dx.astype(np.float32)
    dy = dy.astype(np.float32)
    dist = np.sqrt(dx * dx + dy * dy + np.float32(1e-8))
    theta = np.arctan2(dy, dx)
    rot_in = np.stack([dist, np.sin(theta), np.cos(theta)], axis=-1)
    trans_in = np.stack([dx, dy], axis=-1)
    refl_in = np.concatenate([trans_in, -trans_in], axis=-1)

    def mlp(inp, w1, b1, w2):
        h = np.maximum(inp @ w1 + b1, 0.0)
        return h @ w2

    out = (mlp(rot_in, rot_w1, rot_b1, rot_w2)
           + mlp(trans_in, trans_w1, trans_b1, trans_w2)
           + mlp(refl_in, refl_w1, refl_b1, refl_w2))
    return out.astype(np.float32)


def _expand_idx():
    """idx[i, j] -> difference-class index into the flat 63x63 table."""
    i = np.arange(S)
    ai, bi = i // G, i % G
    da = ai[None, :] - ai[:, None] + (G - 1)
    db = bi[None, :] - bi[:, None] + (G - 1)
    return (da * NDIFF + db).astype(np.int32)


_IDX = None


def _host_bias_rows(inputs, layer):
    """Full bias rows [B, S, S] float32 for one layer."""
    global _IDX
    args = (inputs["rot_w1"][layer], inputs["rot_b1"][layer],
            inputs["rot_w2"][layer],
            inputs["trans_w1"][layer], inputs["trans_b1"][layer],
            inputs["trans_w2"][layer],
            inputs["refl_w1"][layer], inputs["refl_b1"][layer],
            inputs["refl_w2"][layer])
    coords = np.asarray(inputs["coords"], np.float32)
    if np.array_equal(coords, _grid_coords_np()):
        d = (np.arange(NDIFF, dtype=np.float64) - (G - 1)) / (G - 1)
        dxg, dyg = np.meshgrid(d, d, indexing="ij")
        tab = _pair_bias_from_rel(dxg, dyg, *args).ravel()
        if _IDX is None:
            _IDX = _expand_idx()
        full = tab[_IDX]
        return np.broadcast_to(full[None], (B, S, S))
    out = np.empty((B, S, S), np.float32)
    for b in range(B):
        cb = coords[b]
        dx = cb[None, :, 0] - cb[:, None, 0]
        dy = cb[None, :, 1] - cb[:, None, 1]
        out[b] = _pair_bias_from_rel(dx, dy, *args)
    return out


def _pack(arr, k, n):
    """[k*128, n] -> [128, k*n]: chunk c's rows land at cols [c*n,(c+1)*n)."""
    return np.ascontiguousarray(
        arr.reshape(k, 128, n).transpose(1, 0, 2).reshape(128, k * n))


def _pack_pairs(arr, nblk):
    """[512 rows, nblk*128 cols] -> [128, nblk * 2 * 2 * 128] laid out as
    [p][blk][pair][two][m]: dual-fp8 lhsT pair halves contiguous."""
    a = arr.reshape(2, 2, 128, nblk, 128)        # (pair, two, p, blk, m)
    return np.ascontiguousarray(
        a.transpose(2, 3, 0, 1, 4).reshape(128, -1))


# ----------------------------------------------------------------------------
# device program: one transformer layer for 256 query rows of one batch
# ----------------------------------------------------------------------------

def _build_program():
    import concourse.mybir as mybir
    import concourse.tile as tile
    from concourse import bacc

    F32 = mybir.dt.float32
    BF = mybir.dt.bfloat16
    FP8 = mybir.dt.float8e4
    AF = mybir.ActivationFunctionType
    ALU = mybir.AluOpType
    DR = mybir.MatmulPerfMode.DoubleRow

    nc = bacc.Bacc()

    def din(name, shape, dt=BF):
        return nc.dram_tensor(name, shape, dt, kind="ExternalInput")

    # packed layouts: [128, chunks * width]
    xt2 = din("xt2", [128, KD * S], FP8)  # x^T chunks (d-major), fp8
    xtr2 = din("xtr2", [128, KD * R], FP8)  # x^T, this core's query columns
    xrow2 = din("xrow2", [128, NJT * D], FP8)  # x row chunks, fp8
    xr2 = din("xr2", [128, NIT * D], F32)  # residual rows (fp32)
    sxr = din("sxr", [128, NIT], F32)     # row-sums of xr (for LN1 mean)
    m2 = din("m2", [128, KD * D], FP8)    # (Wq @ Wk^T) * 64, fp8
    wv2 = din("wv2", [128, KD * D])
    bias2 = din("bias2", [128, NJT * R])  # pairwise bias^T * 64
    fw1x = din("fw1x", [128, KD * F])
    fb1t = din("fb1t", [128, KF], F32)
    fw2x = din("fw2x", [128, KF * D])
    fb2x = din("fb2x", [1, D])            # ffn_b2 + ln1_b (host-folded)
    ln1bt = din("ln1bt", [128, KD], F32)  # ln1_b chunked per-partition
    ln1g = din("ln1g", [1, D], F32)
    idd = din("idd", [128, 128])          # bf16 identity

    xout = nc.dram_tensor("xout", [R, D], F32, kind="ExternalOutput")

    inv_scale = 1.0 / math.sqrt(D)

    def mm(out, lhsT, rhs, start, stop, perf_mode=None):
        nc.tensor.matmul(out, lhsT, rhs, start=start, stop=stop,
                         perf_mode=perf_mode)

    with tile.TileContext(nc) as tc:
        from contextlib import ExitStack
        es = ExitStack()
        with es:
            p_const = es.enter_context(tc.tile_pool(name="const", bufs=1))
            # PSUM banks: mm 3 + sc 3 + tp 2 = 8
            p_mm = es.enter_context(
                tc.tile_pool(name="pmm", bufs=3, space="PSUM"))
            p_sc = es.enter_context(
                tc.tile_pool(name="psc", bufs=3, space="PSUM"))
            p_tp = es.enter_context(
                tc.tile_pool(name="ptp", bufs=2, space="PSUM"))

            p_sb = es.enter_context(tc.tile_pool(name="sb", bufs=1))
            p_tmp = es.enter_context(tc.tile_pool(name="tmp", bufs=2))

            ones_c = p_const.tile([128, 1], FP8, tag="ones_c",
                                  name="ones_c")
            nc.vector.memset(ones_c[:], 1.0)
            ones_r = p_const.tile([1, 128], BF, tag="ones_r", name="ones_r")
            nc.vector.memset(ones_r[:], 1.0)
            ones_1 = p_const.tile([1, 1], F32, tag="ones_1", name="ones_1")
            nc.vector.memset(ones_1[:], 1.0)
            eps_t = p_const.tile([128, 1], F32, tag="eps", name="eps")
            nc.vector.memset(eps_t[:], EPS_LN)

            # ---- persistent SBUF tiles (packed, sliced per chunk) -----
            # lhsT operands for dual-fp8 LDWEIGHTS need the pair halves
            # contiguous: [.., pair, block, two, 128] layouts
            XT = p_sb.tile([128, NJT, KD // 2, 2, 128], FP8,
                           tag="xt", name="xt")
            XTR = p_sb.tile([128, KD, R], FP8, tag="xtr", name="xtr")
            XRW = p_sb.tile([128, NJT, D], FP8, tag="xrw", name="xrw")
            XR = p_sb.tile([128, NIT * D], F32, tag="xr", name="xr")
            SXR = p_sb.tile([128, NIT], F32, tag="sxr", name="sxr")
            MM2 = p_sb.tile([128, KD, KD // 2, 2, 128], FP8,
                            tag="m2", name="m2")
            WV = p_sb.tile([128, KD * D], BF, tag="wv", name="wv")
            BIA = p_sb.tile([128, NJT * R], BF, tag="bia", name="bia")
            FW1 = p_sb.tile([128, KD, F], BF, tag="fw1", name="fw1")
            FW2 = p_sb.tile([128, KF, D], BF, tag="fw2", name="fw2")
            P = p_sb.tile([128, KD, R], FP8, tag="p", name="p")
            EE = p_sb.tile([128, NJT // 2, NIT, 2, 128], FP8,
                           tag="ee", name="ee")
            AXN = [p_sb.tile([128, D], BF, tag=f"axn{i}", name=f"axn{i}")
                   for i in range(NIT)]
            AXT = p_sb.tile([128, KD * R], BF, tag="axt", name="axt")
            RZR = p_sb.tile([1, R], F32, tag="rzr", name="rzr")
            RZC = [p_sb.tile([128, 1], F32, tag=f"rz{i}", name=f"rz{i}")
                   for i in range(NIT)]
            XN1 = [p_sb.tile([128, D], BF, tag=f"xn1_{i}", name=f"xn1_{i}")
                   for i in range(NIT)]
            XNT = p_sb.tile([128, KD, R], BF, tag="xnt", name="xnt")
            H1T = p_sb.tile([128, KF, R], BF, tag="h1t", name="h1t")
            XO = [p_sb.tile([128, D], F32, tag=f"xo{i}", name=f"xo{i}")
                  for i in range(NIT)]

            def sl(tile_, c, w, o0=0, ow=None):
                base = c * w + o0
                return tile_[:, base: base + (ow if ow is not None else w)]

            # ---- DMA: sync queue carries the early critical path ------
            nc.sync.dma_start(MM2[:], m2[:])
            nc.sync.dma_start(XTR[:], xtr2[:])
            nc.sync.dma_start(XT[:], xt2[:])
            nc.sync.dma_start(XRW[:], xrow2[:])
            nc.sync.dma_start(WV[:], wv2[:])

            # scalar queue: only the bias before ACT's compute section
            nc.scalar.dma_start(BIA[:], bias2[:])

            # gpsimd SWDGE: small constants
            iddt = p_const.tile([128, 128], BF, tag="idd", name="idd")
            nc.gpsimd.dma_start(iddt[:], idd[:])
            fb2t = p_const.tile([1, D], BF, tag="fb2", name="fb2")
            nc.gpsimd.dma_start(fb2t[:], fb2x[:])
            fb1tt = p_const.tile([128, KF], F32, tag="fb1t", name="fb1t")
            nc.gpsimd.dma_start(fb1tt[:], fb1t[:])
            l1bt = p_const.tile([128, KD], F32, tag="l1bt", name="l1bt")
            nc.gpsimd.dma_start(l1bt[:], ln1bt[:])
            nc.gpsimd.dma_start(SXR[:], sxr[:])
            lnp = {}
            for nm, tsr in (("ln1g", ln1g),):
                row = p_const.tile([1, D], F32, tag=nm + "_r")
                nc.gpsimd.dma_start(row[:], tsr[:])
                bc = p_const.tile([128, D], F32, tag=nm + "_b")
                nc.gpsimd.partition_broadcast(bc[:], row[:])
                lnp[nm] = bc

            # ---- P = 64 * M^T x_r^T / sqrt(D)  [D, R], fp8 ------------
            # (m2 is pre-scaled by 64 on host; psum = 64 M^T x_r^T)
            for e in range(KD):
                ps = p_sc.tile([128, R], F32, tag="sc", name="sc")
                for c in range(KD // 2):
                    mm(ps[:], MM2[:, e, c, :, :],
                       XTR[:, 2 * c:2 * c + 2, :],
                       c == 0, c == KD // 2 - 1, perf_mode=DR)
                nc.scalar.activation(P[:, e, :], ps[:], AF.Copy,
                                     scale=inv_scale)

            # ---- scoresT[j, i] * 64 = x @ P + 64*bias^T; exp ----------
            for j in range(NJT):
                ps = p_sc.tile([128, R], F32, tag="sc", name="sc")
                for c in range(KD // 2):
                    mm(ps[:], XT[:, j, c, :, :],
                       P[:, 2 * c:2 * c + 2, :],
                       c == 0, c == KD // 2 - 1, perf_mode=DR)
                nc.vector.tensor_tensor(ps[:], ps[:], sl(BIA, j, R), ALU.add)
                nc.scalar.activation(EE[:, j // 2, :, j % 2, :], ps[:],
                                     AF.Exp, scale=1.0 / SC_SCALE)

            # late bulk loads, issued from the scalar queue AFTER the
            # exp instructions so they cannot head-of-line block ACT
            nc.scalar.dma_start(XR[:], xr2[:])
            nc.scalar.dma_start(FW1[:], fw1x[:])
            nc.scalar.dma_start(FW2[:], fw2x[:])

            # ---- AX[i, :] = sum_j ee[j, i] * x[j, :]  (fp8 DR) --------
            for it in range(NIT):
                ps = p_mm.tile([128, D], F32, tag="mm", name="mm")
                for c in range(NJT // 2):
                    mm(ps[:], EE[:, c, it, :, :],
                       XRW[:, 2 * c:2 * c + 2, :],
                       c == 0, c == NJT // 2 - 1, perf_mode=DR)
                nc.vector.tensor_copy(AXN[it][:], ps[:])

            # ---- Z row = ones^T @ EE; rz = 1/Z; spread to columns -----
            zs = p_tp.tile([1, R], F32, tag="tp", name="tp")
            for j in range(NJT):
                mm(zs[:], ones_c[:], EE[:, j // 2, :, j % 2, :],
                   j == 0, j == NJT - 1)
            nc.vector.reciprocal(RZR[:], zs[:])
            for it in range(NIT):
                cs = p_tp.tile([128, 1], F32, tag="tp", name="tp")
                mm(cs[:], RZR[:, 128 * it:128 * (it + 1)], ones_1[:],
                   True, True)
                nc.vector.tensor_copy(RZC[it][:], cs[:])

            # ---- transpose AX -> AXT; attn_out = AX @ Wv; LN1 ---------
            for it in range(NIT):
                for e in range(KD):
                    tp = p_tp.tile([128, 128], BF, tag="tp", name="tp")
                    nc.tensor.transpose(
                        tp[:], AXN[it][:, 128 * e:128 * (e + 1)], iddt[:])
                    nc.vector.tensor_copy(
                        sl(AXT, e, R, 128 * it, 128), tp[:])
                ps = p_mm.tile([128, D], F32, tag="mm", name="mm")
                for e in range(KD):
                    mm(ps[:], sl(AXT, e, R, 128 * it, 128), sl(WV, e, D),
                       e == 0, e == KD - 1)
                # t = rz * attn_out (ACT evac), s1 = row-sum(t)
                t = p_tmp.tile([128, D], F32, tag="t", name="t")
                s1 = p_tmp.tile([128, 1], F32, tag="s1", name="s1")
                nc.scalar.activation(t[:], ps[:], AF.Copy,
                                     scale=RZC[it][:], accum_out=s1[:])
                # mu_neg = -(s1 + sxr)/D
                mun = p_tmp.tile([128, 1], F32, tag="mun", name="mun")
                nc.vector.tensor_tensor(mun[:], s1[:],
                                        SXR[:, it:it + 1], ALU.add)
                nc.vector.tensor_scalar_mul(mun[:], mun[:], -1.0 / D)
                # zc = (t + mu_neg) + xr
                zc = p_tmp.tile([128, D], F32, tag="zc", name="zc")
                nc.vector.scalar_tensor_tensor(
                    zc[:], t[:], mun[:], sl(XR, it, D), ALU.add, ALU.add)
                # var, rstd
                s2 = p_tmp.tile([128, 1], F32, tag="s2", name="s2")
                nc.scalar.activation(t[:], zc[:], AF.Square, accum_out=s2[:])
                std = p_tmp.tile([128, 1], F32, tag="std", name="std")
                nc.scalar.activation(std[:], s2[:], AF.Sqrt,
                                     scale=1.0 / D, bias=eps_t[:])
                rstd = p_tmp.tile([128, 1], F32, tag="rstd", name="rstd")
                nc.vector.reciprocal(rstd[:], std[:])
                # xn1 = zc * rstd * g   (beta folded downstream)
                nc.vector.scalar_tensor_tensor(
                    XN1[it][:], zc[:], rstd[:], lnp["ln1g"][:],
                    ALU.mult, ALU.mult)

            # ---- transpose XN1 -> XNT (+ln1_b on evacuation) ----------
            for it in range(NIT):
                for d in range(KD):
                    tp = p_tp.tile([128, 128], BF, tag="tp", name="tp")
                    nc.tensor.transpose(
                        tp[:], XN1[it][:, 128 * d:128 * (d + 1)], iddt[:])
                    nc.scalar.activation(
                        XNT[:, d, 128 * it:128 * (it + 1)], tp[:],
                        AF.Identity, bias=l1bt[:, d:d + 1])

            # ---- FFN (bf16) -------------------------------------------
            for f in range(KF):
                ps = p_sc.tile([128, R], F32, tag="sc", name="sc")
                for d in range(KD):
                    mm(ps[:], FW1[:, d, 128 * f:128 * (f + 1)],
                       XNT[:, d, :], d == 0, d == KD - 1)
                nc.scalar.activation(H1T[:, f, :], ps[:], AF.Relu,
                                     bias=fb1tt[:, f:f + 1])

            for it in range(NIT):
                ps = p_mm.tile([128, D], F32, tag="mm", name="mm")
                # preload (ffn_b2 + ln1_b) broadcast + residual xn1
                nc.tensor.matmul(ps[:], ones_r[:], fb2t[:],
                                 start=True, stop=False)
                nc.tensor.matmul(ps[:], iddt[:], XN1[it][:],
                                 start=False, stop=False)
                for f in range(KF):
                    mm(ps[:], H1T[:, f, 128 * it:128 * (it + 1)],
                       FW2[:, f, :], False, f == KF - 1)
                # z2 ships to host; LN2 runs there
                nc.scalar.activation(XO[it][:], ps[:], AF.Copy)
                nc.sync.dma_start(xout[128 * it:128 * (it + 1), :],
                                  XO[it][:])

    nc.compile()
    return nc


def _get_program():
    global _prog
    if _prog is None:
        _prog = _build_program()
    return _prog


# ----------------------------------------------------------------------------
# host glue
# ----------------------------------------------------------------------------

_exec = None        # cached (jitted_fn, in_names, out_names, out_avals, mesh)


def _get_exec(nc):
    """Build the PJRT executable once (run_bass_via_pjrt rebuilds its jit on
    every call, costing seconds of retrace; this is the same lowering with
    the jit cached)."""
    global _exec
    if _exec is not None:
        return _exec
    import jax
    import numpy as np_
    from jax.sharding import Mesh, PartitionSpec
    from jax.experimental.shard_map import shard_map
    import concourse.mybir as mybir
    from concourse.bass2jax import (_bass_exec_p, install_neuronx_cc_hook,
                                    partition_id_tensor)

    install_neuronx_cc_hook()
    partition_name = (nc.partition_id_tensor.name
                      if nc.partition_id_tensor else None)
    in_names, out_names, out_avals = [], [], []
    for alloc in nc.m.functions[0].allocations:
        if not isinstance(alloc, mybir.MemoryLocationSet):
            continue
        name = alloc.memorylocations[0].name
        if alloc.kind == "ExternalInput":
            if name != partition_name:
                in_names.append(name)
        elif alloc.kind == "ExternalOutput":
            out_names.append(name)
            out_avals.append(jax.core.ShapedArray(
                tuple(alloc.tensor_shape), mybir.dt.np(alloc.dtype)))
    n_params = len(in_names)
    n_outs = len(out_names)
    all_names = in_names + out_names
    if partition_name is not None:
        all_names.append(partition_name)
    donate = tuple(range(n_params, n_params + n_outs))

    def _body(*args):
        operands = list(args)
        if partition_name is not None:
            operands.append(partition_id_tensor())
        return tuple(_bass_exec_p.bind(
            *operands,
            out_avals=tuple(out_avals),
            in_names=tuple(all_names),
            out_names=tuple(out_names),
            lowering_input_output_aliases=(),
            sim_require_finite=True,
            sim_require_nnan=True,
            nc=nc,
        ))

    devices = jax.devices()[:NCORES]
    mesh = Mesh(np_.asarray(devices), ("core",))
    core_spec = PartitionSpec("core")
    repl_spec = PartitionSpec()
    in_specs = tuple(core_spec if n in _VARYING else repl_spec
                     for n in in_names) + (core_spec,) * n_outs
    fn = jax.jit(
        shard_map(_body, mesh=mesh,
                  in_specs=in_specs,
                  out_specs=(core_spec,) * n_outs,
                  check_rep=False),
        donate_argnums=donate, keep_unused=True)
    _exec = (fn, in_names, out_names, out_avals, mesh)
    return _exec


_VARYING = {"xt2", "xtr2", "xrow2", "xr2", "sxr", "bias2"}
_repl_cache = {}


def _repl_device_put(name, arr, mesh):
    """Upload a replicated input once; reuse device array on same content."""
    import hashlib
    import jax
    from jax.sharding import NamedSharding, PartitionSpec
    key = (name, arr.shape, hashlib.blake2b(arr.tobytes(),
                                            digest_size=16).digest())
    hit = _repl_cache.get(key)
    if hit is not None:
        return hit
    dev = jax.device_put(arr, NamedSharding(mesh, PartitionSpec()))
    _repl_cache[key] = dev
    if len(_repl_cache) > 64:
        _repl_cache.pop(next(iter(_repl_cache)))
    return dev


def _run_fast(nc, in_maps):
    fn, in_names, out_names, out_avals, mesh = _get_exec(nc)
    args = []
    for n in in_names:
        if n in _VARYING:
            args.append(np.concatenate([m[n] for m in in_maps], axis=0))
        else:
            args.append(_repl_device_put(n, in_maps[0][n], mesh))
    zeros = [np.zeros((NCORES * a.shape[0], *a.shape[1:]), a.dtype)
             for a in out_avals]
    outs = fn(*args, *zeros)
    res = []
    for c in range(NCORES):
        res.append({n: np.asarray(outs[i]).reshape(
            NCORES, *out_avals[i].shape)[c]
            for i, n in enumerate(out_names)})
    return res


def _launch(nc, x, bias_rows, inputs, layer, trace=False):
    """One transformer layer across 8 cores. Returns (x_next, None, results)."""
    from concourse.bass_utils import run_bass_kernel_spmd

    idd = np.eye(128, dtype=BF16)
    fb1t = np.ascontiguousarray(
        inputs["ffn_b1"][layer].reshape(KF, 128).T.astype(np.float32))
    ln1b = inputs["ln1_b"][layer].astype(np.float32)
    fb2x = (inputs["ffn_b2"][layer].astype(np.float32)
            + ln1b).reshape(1, D).astype(BF16)
    ln1bt = np.ascontiguousarray(
        ln1b.reshape(KD, 128).T.astype(np.float32))
    M = (inputs["Wq"][layer] @ inputs["Wk"][layer].T).astype(np.float32)

    m2 = _pack_pairs((SC_SCALE * M).astype(F8), KD)
    wv2 = _pack(inputs["Wv"][layer].astype(BF16), KD, D)
    fw1x = _pack(inputs["ffn_w1"][layer].astype(BF16), KD, F)
    fw2x = _pack(inputs["ffn_w2"][layer].astype(BF16), KF, D)

    xT_b, xrow_b = [], []
    for b in range(B):
        xTb = np.ascontiguousarray(x[b].T.astype(F8))
        xT_b.append(xTb)
        xrow_b.append(_pack(x[b].astype(F8), NJT, D))
    xt2_b = [_pack_pairs(xTb, NJT) for xTb in xT_b]

    in_maps = []
    for core in range(NCORES):
        b, q = divmod(core, QB)
        r0 = q * R
        xr = np.ascontiguousarray(x[b][r0:r0 + R]).astype(np.float32)
        mp = {
            "xt2": xt2_b[b],
            "xtr2": _pack(np.ascontiguousarray(xT_b[b][:, r0:r0 + R]),
                          KD, R),
            "xrow2": xrow_b[b],
            "xr2": _pack(xr, NIT, D),
            "sxr": np.ascontiguousarray(
                xr.sum(axis=1).reshape(NIT, 128).T.astype(np.float32)),
            "m2": m2,
            "wv2": wv2,
            "bias2": _pack((SC_SCALE * np.ascontiguousarray(
                bias_rows[b][r0:r0 + R].T)).astype(BF16), NJT, R),
            "fw1x": fw1x,
            "fb1t": fb1t,
            "fw2x": fw2x,
            "fb2x": fb2x,
            "ln1bt": ln1bt,
            "ln1g": inputs["ln1_g"][layer].reshape(1, D).astype(np.float32),
            "idd": idd,
        }
        in_maps.append({k: np.ascontiguousarray(v) for k, v in mp.items()})

    if trace:
        res = run_bass_kernel_spmd(nc, in_maps, list(range(NCORES)),
                                   trace=True)
        outs = res.results
    else:
        res = None
        outs = _run_fast(nc, in_maps)
    z2 = np.empty((B, S, D), np.float32)
    for core in range(NCORES):
        b, q = divmod(core, QB)
        z2[b, q * R:(q + 1) * R] = outs[core]["xout"]
    # LN2 on host (device ships the pre-norm residual stream)
    g = inputs["ln2_g"][layer].astype(np.float32)
    bb = inputs["ln2_b"][layer].astype(np.float32)
    mu = z2.mean(-1, keepdims=True)
    var = ((z2 - mu) ** 2).mean(-1, keepdims=True)
    x_next = (z2 - mu) / np.sqrt(var + EPS_LN) * g + bb
    return x_next.astype(np.float32), None, res


def _host_head(x, inputs):
    """Final LN + mean pool + fc on host."""
    g = inputs["lnf_g"].astype(np.float32)
    bb = inputs["lnf_b"].astype(np.float32)
    mu = x.mean(-1, keepdims=True)
    var = ((x - mu) ** 2).mean(-1, keepdims=True)
    xn = (x - mu) / np.sqrt(var + EPS_LN) * g + bb
    pooled = xn.mean(axis=1)
    return pooled @ inputs["fc_w"].astype(np.float32) \
        + inputs["fc_b"].astype(np.float32)[None, :]


def kernel(**inputs):
    inputs = {k: np.asarray(v) for k, v in inputs.items()}
    nc = _get_program()
    x = np.asarray(inputs["x"], np.float32)
    for layer in range(L):
        bias_rows = _host_bias_rows(inputs, layer)
        x, _, _ = _launch(nc, x, bias_rows, inputs, layer)
    out = _host_head(x, inputs)
    return out.astype(np.float32)
